# revision 17
# baseline (speedup 1.0000x reference)
"""AttentiveLISTA Trainium2 kernel — pure data parallel over 8 NeuronCores.

Per core (B=1 image, C=64, H=W=64, A=256):
  conv3x3(64->256)+bias -> 2x residual blocks -> CBAM (channel+spatial
  attention) -> per-pixel LISTA (z0 = soft(y); 16x z = soft(z@S + y/L)).

Implementation notes:
  * channels on SBUF partitions, flattened padded 66x66 image on the free dim
    (base offset +2, buffers [*, 4360]); 3x3 convs = shifted matmuls (conv1
    pairs taps kh=0/kh=1 per K=128 chunk via a second +66-shifted x copy).
  * The conv/CBAM tower only produces the soft-thresholds lam (the LISTA data
    path is x/Dict/S alone), so the tower runs in bf16: same 1 cyc/row PE
    speed, half the SBUF, and a ~1e-2-relative lam error that perturbs z by
    ~|dlam| ~ 3e-5 absolute.
  * LISTA matmuls in float32r (~13-bit-mantissa fp32, 1 PE cycle/row).
  * LISTA soft-threshold = one fused custom DVE op per tile:
        out = v > 0 ? relu(v - lam) : -relu(-v - lam)
    reading v straight from PSUM, writing z as f32r (matmul-ready).
  * y/L rides the loop's PSUM accumulation as a 3rd K=64 matmul (Dict/L @ x),
    so no separate add pass exists.
"""
import os
import sys

for _p in ("/opt/trn_rl_repo", "/opt/trn_rl_repo/concourse"):
    if _p not in sys.path:
        sys.path.insert(0, _p)

import numpy as np
import ml_dtypes

import concourse.bass as bass
from concourse import bacc, mybir, bass_isa
import concourse.tile as tile
from concourse.bass_utils import run_bass_kernel_spmd

f32 = mybir.dt.float32
f32r = mybir.dt.float32r
bf16 = mybir.dt.bfloat16
Alu = mybir.AluOpType
Act = mybir.ActivationFunctionType

W_PAD = 66           # padded row width
BASE = 2             # flat offset of padded pixel 0 inside the [*, 4360] bufs
BUFW = 4360
INT0 = BASE + W_PAD  # 68: first interior-row flat position
INTN = 64 * W_PAD    # 4224: contiguous interior span (rows 1..64 incl. side pads)
NPIX = 4096

_SOFT_OP = None


def _register_softshrink():
    global _SOFT_OP
    if _SOFT_OP is not None:
        return _SOFT_OP
    import concourse.dve_ops as dve_ops
    from concourse.dve_spec import Spec, Src0, Src1, Zero, relu, select, lower
    from concourse.dve_uop import DveOpSpec

    name = "SOFTSHRINK_ANT"
    for op in dve_ops.OPS:
        if op.name == name:
            _SOFT_OP = op
            return op

    def _reference(in0, in1, s0, s1, imm2):
        in0 = in0.astype(np.float32)
        return np.where(in0 > 0, np.maximum(in0 - in1, 0.0),
                        -np.maximum(-in0 - in1, 0.0)).astype(np.float32)

    body = select(Src0 > Zero, relu(Src0 - Src1),
                  Zero - relu(Zero - Src0 - Src1))
    spec = Spec(body=body, reference=_reference)
    row = dve_ops._CUSTOM_DVE_ROW_BASE + len(dve_ops.OPS)
    assert row < 0x20
    shas = {}
    for ver in ("v3", "v4"):
        tmp = DveOpSpec(name=name, opcode=row, uops=lower(spec, ver=ver),
                        rd1_en=True)
        shas[ver] = tmp.sha(ver)
    op = dve_ops.DveOp(name, spec, subdim=False, uops_sha=shas)
    dve_ops.OPS.append(op)
    dve_ops.CUSTOM_DVE_SPECS[name] = spec
    dve_ops._SUB_OPCODE_FOR_NAME[name] = row
    _SOFT_OP = op
    return op


def _chunks512(start, span):
    out = []
    o = start
    while o < start + span:
        n = min(512, start + span - o)
        out.append((o, n))
        o += n
    return out


def build_nc(num_iters, L, sa_w):
    SOFT = _register_softshrink()
    nc = bacc.Bacc("TRN2", target_bir_lowering=False, debug=False)

    x_d = nc.dram_tensor("x", [64, 64, 64], f32, kind="ExternalInput")
    cwp_d = nc.dram_tensor("cwp", [128, 2, 3, 128], f32, kind="ExternalInput")
    cws_d = nc.dram_tensor("cws", [64, 2, 3, 128], f32, kind="ExternalInput")
    b2_d = nc.dram_tensor("b2", [128, 2], f32, kind="ExternalInput")
    rw1a_d = nc.dram_tensor("rw1a", [128, 9, 2, 128], bf16, kind="ExternalInput")
    rw2a_d = nc.dram_tensor("rw2a", [128, 256], bf16, kind="ExternalInput")
    rw1b_d = nc.dram_tensor("rw1b", [128, 9, 2, 128], bf16, kind="ExternalInput")
    rw2b_d = nc.dram_tensor("rw2b", [128, 256], bf16, kind="ExternalInput")
    caw1_d = nc.dram_tensor("caw1", [128, 2, 16], f32, kind="ExternalInput")
    caw2_d = nc.dram_tensor("caw2", [16, 256], f32, kind="ExternalInput")
    s_d = nc.dram_tensor("S", [128, 2, 256], f32, kind="ExternalInput")
    dict_d = nc.dram_tensor("Dct", [64, 256], f32, kind="ExternalInput")
    ident_d = nc.dram_tensor("ident", [128, 128], f32, kind="ExternalInput")
    dictt_d = nc.dram_tensor("DctT", [128, 2, 64], f32, kind="ExternalInput")

    zo_d = nc.dram_tensor("zo", [256, NPIX], f32, kind="ExternalOutput")
    ro_d = nc.dram_tensor("ro", [64, NPIX], f32, kind="ExternalOutput")
    sab_d = nc.dram_tensor("sab", [2, 4096], f32)
    ssb_d = nc.dram_tensor("ssb", [1, 4096], f32)

    conv_chunks = _chunks512(INT0, INTN)

    with tile.TileContext(nc) as tc:
        wp_cm = tc.tile_pool(name="wp", bufs=1)
        ev_cm = tc.tile_pool(name="ev", bufs=4)
        pp_cm = tc.tile_pool(name="pp", bufs=8, space="PSUM")
        zp_cm = tc.tile_pool(name="zp", bufs=3)
        late_cm = tc.tile_pool(name="late", bufs=1)
        cw_cm = tc.tile_pool(name="cw", bufs=1)
        t2_cm = tc.tile_pool(name="t2", bufs=1)
        t1_cm = tc.tile_pool(name="t1", bufs=1)
        x2_cm = tc.tile_pool(name="x2p", bufs=1)
        wp = wp_cm.__enter__()
        ev = ev_cm.__enter__()
        pp = pp_cm.__enter__()
        zp = zp_cm.__enter__()
        late = late_cm.__enter__()
        cw = cw_cm.__enter__()
        t2 = t2_cm.__enter__()
        t1 = t1_cm.__enter__()

        # ---- long-lived small weights ------------------------------------
        b2 = wp.tile([128, 2], f32, name="b2")
        nc.sync.dma_start(b2[:], b2_d[:])
        caw1 = wp.tile([128, 2, 16], f32r, name="caw1")
        nc.sync.dma_start(caw1[:], caw1_d[:].bitcast(f32r))
        caw2 = wp.tile([16, 256], f32r, name="caw2")
        nc.sync.dma_start(caw2[:], caw2_d[:].bitcast(f32r))
        s_t = wp.tile([128, 2, 256], f32r, name="s_t")
        nc.sync.dma_start(s_t[:], s_d[:].bitcast(f32r))
        dctF = wp.tile([64, 256], f32, name="dctF")
        nc.sync.dma_start(dctF[:], dict_d[:])
        ident = wp.tile([128, 128], f32r, name="ident")
        nc.sync.dma_start(ident[:], ident_d[:].bitcast(f32r))
        dctt = wp.tile([128, 2, 64], f32r, name="dctt")
        nc.sync.dma_start(dctt[:], dictt_d[:].bitcast(f32r))
        ones1 = wp.tile([1, 128], f32r, name="ones1")
        nc.vector.memset(ones1.bitcast(f32)[:], 1.0)
        xF = wp.tile([64, NPIX], f32, name="xF")
        nc.sync.dma_start(xF[:], x_d[:].rearrange("c h w -> c (h w)"))

        # lam-path carrier: l -> l1 -> l2, updated in place (compact f32r)
        lc = [late.tile([128, NPIX], f32r, name=f"lc{h}") for h in range(2)]
        yLr = [late.tile([128, NPIX], f32r, name=f"yLr{h}") for h in range(2)]

        # ---- conv1 weights (f32r) + res weights (bf16) --------------------
        cwp = cw.tile([128, 2, 3, 128], f32r, name="cwp")
        nc.sync.dma_start(cwp[:], cwp_d[:].bitcast(f32r))
        cws = cw.tile([64, 2, 3, 128], f32r, name="cws")
        nc.sync.dma_start(cws[:], cws_d[:].bitcast(f32r))
        rw1 = cw.tile([128, 9, 2, 128], bf16, tag="rw1", name="rw1")
        nc.sync.dma_start(rw1[:], rw1a_d[:])
        rw2 = cw.tile([128, 256], bf16, tag="rw2", name="rw2")
        nc.sync.dma_start(rw2[:], rw2a_d[:])

        x2p = x2_cm.__enter__()
        x2 = x2p.tile([128, BUFW], f32r, name="x2")
        nc.vector.memset(x2.bitcast(f32)[0:64, :], 0.0)
        nc.gpsimd.memset(x2.bitcast(f32)[64:128, :], 0.0)
        dst0 = x2[0:64, INT0:INT0 + INTN].rearrange("p (r c) -> p r c", c=W_PAD)[:, :, 1:65]
        nc.sync.dma_start(dst0, x_d[:].bitcast(f32r))
        dst1 = x2[64:128, BASE:BASE + INTN].rearrange("p (r c) -> p r c", c=W_PAD)[:, :, 1:65]
        nc.sync.dma_start(dst1, x_d[:].bitcast(f32r))

        def win3d(buf, j, d=0):
            base = INT0 + j * 8 * W_PAD + d
            return buf[:, base:base + 8 * W_PAD] \
                .rearrange("p (r c) -> p r c", c=W_PAD)[:, :, 1:65]

        def win3d64(buf, j, parts=128):
            base = INT0 + j * 8 * W_PAD
            return buf[0:parts, base:base + 8 * W_PAD] \
                .rearrange("p (r c) -> p r c", c=W_PAD)[:, :, 1:65]

        # ---- conv1 (row-aligned): lc = W@x + b (f32r), rl (bf16 padded) ---
        rl_t = [t1.tile([128, BUFW], bf16, name=f"rlA{h}") for h in range(2)]
        for h in range(2):
            for j in range(8):
                ps = pp.tile([128, 512], f32, tag="ps", name="ps")
                for dc in range(3):
                    nc.tensor.matmul(ps[:], cwp[:, h, dc, :],
                                     win3d(x2, j, dc - 67),
                                     start=(dc == 0), stop=False)
                for dc in range(3):
                    base = INT0 + j * 8 * W_PAD + 65 + dc
                    rhs = x2[0:64, base:base + 8 * W_PAD] \
                        .rearrange("p (r c) -> p r c", c=W_PAD)[:, :, 1:65]
                    nc.tensor.matmul(ps[:], cws[:, h, dc, :], rhs,
                                     start=False, stop=(dc == 2))
                nc.scalar.activation(lc[h][:, j * 512:(j + 1) * 512], ps[:],
                                     Act.Identity, bias=b2[:, h:h + 1])
                nc.vector.tensor_scalar(
                    win3d(rl_t[h], j), ps[:].rearrange("p (r c) -> p r c", c=64),
                    b2[:, h:h + 1], 0.0, op0=Alu.add, op1=Alu.max)
        for h in range(2):
            nc.gpsimd.memset(rl_t[h][:, 0:INT0], 0.0)
            nc.gpsimd.memset(rl_t[h][:, INT0 + INTN:BUFW], 0.0)
            side = rl_t[h][:, INT0:INT0 + INTN].rearrange("p (r c) -> p r c", c=W_PAD)
            nc.gpsimd.memset(side[:, :, 0:1], 0.0)
            nc.gpsimd.memset(side[:, :, 65:66], 0.0)
        x2_cm.__exit__(None, None, None)

        def conv3x3(rin, rw, tname):
            rt = t2.tile([128, BUFW], bf16, tag="rt", name=tname)
            for ci, (off, n) in enumerate(conv_chunks):
                ps = pp.tile([128, 512], f32, tag="ps", name="ps")
                k = 0
                for kh in range(3):
                    for kw in range(3):
                        d = (kh - 1) * W_PAD + (kw - 1)
                        for h in range(2):
                            nc.tensor.matmul(ps[:, :n], rw[:, kh * 3 + kw, h, :],
                                             rin[h][:, off + d: off + d + n],
                                             start=(k == 0), stop=(k == 17))
                            k += 1
                if ci % 2 == 0:
                    nc.vector.tensor_scalar(rt[:, off:off + n], ps[:, :n],
                                            0.0, None, op0=Alu.max)
                else:
                    nc.scalar.activation(rt[:, off:off + n], ps[:, :n], Act.Relu)
            return rt

        # ---- res1: lc += conv1x1(rt1); rl1 = relu(lc) ---------------------
        rt1 = conv3x3(rl_t, rw1, "rt1")
        rl1 = [t2.tile([128, BUFW], bf16, name=f"rlB{h}") for h in range(2)]
        for h in range(2):
            for j in range(8):
                ps = pp.tile([128, 512], f32, tag="ps", name="ps")
                nc.tensor.matmul(ps[:], rw2[:, 128 * h:128 * h + 128],
                                 win3d64(rt1, j), start=True, stop=True)
                nc.vector.tensor_tensor(lc[h][:, j * 512:(j + 1) * 512], ps[:],
                                        lc[h][:, j * 512:(j + 1) * 512],
                                        op=Alu.add)
                nc.scalar.activation(
                    win3d(rl1[h], j),
                    lc[h][:, j * 512:(j + 1) * 512].rearrange("p (r c) -> p r c", c=64),
                    Act.Relu)
            nc.gpsimd.memset(rl1[h][:, 0:INT0], 0.0)
            nc.gpsimd.memset(rl1[h][:, INT0 + INTN:BUFW], 0.0)
            side = rl1[h][:, INT0:INT0 + INTN].rearrange("p (r c) -> p r c", c=W_PAD)
            nc.gpsimd.memset(side[:, :, 0:1], 0.0)
            nc.gpsimd.memset(side[:, :, 65:66], 0.0)
        t1_cm.__exit__(None, None, None)

        # res2 weights reuse the res1 slots
        rw1b = cw.tile([128, 9, 2, 128], bf16, tag="rw1", name="rw1b")
        nc.sync.dma_start(rw1b[:], rw1b_d[:])
        rw2b = cw.tile([128, 256], bf16, tag="rw2", name="rw2b")
        nc.sync.dma_start(rw2b[:], rw2b_d[:])

        # ---- res2: lc += conv1x1(rt2), channel sums ride the update -------
        rt2 = conv3x3(rl1, rw1b, "rt2")
        sums = [ev.tile([128, 8], f32, tag=f"sum{h}", name=f"sum{h}", bufs=1)
                for h in range(2)]
        for h in range(2):
            for j in range(8):
                ps = pp.tile([128, 512], f32, tag="ps", name="ps")
                nc.tensor.matmul(ps[:], rw2b[:, 128 * h:128 * h + 128],
                                 win3d64(rt2, j), start=True, stop=True)
                nc.vector.scalar_tensor_tensor(
                    lc[h][:, j * 512:(j + 1) * 512], ps[:], 0.0,
                    lc[h][:, j * 512:(j + 1) * 512], op0=Alu.add, op1=Alu.add,
                    accum_out=sums[h][:, j:j + 1])
        t2_cm.__exit__(None, None, None)
        cw_cm.__exit__(None, None, None)

        # ---- CBAM channel attention --------------------------------------
        avmx = []
        for h in range(2):
            ssum = ev.tile([128, 1], f32, tag="ssum", name="ssum", bufs=2)
            nc.vector.tensor_reduce(ssum[:], sums[h][:], axis=mybir.AxisListType.X,
                                    op=Alu.add)
            am = ev.tile([128, 2], f32r, tag=f"avmx{h}", name=f"avmx{h}", bufs=1)
            nc.vector.tensor_scalar(am[:, 0:1], ssum[:], 1.0 / NPIX, None,
                                    op0=Alu.mult)
            nc.vector.tensor_reduce(am[:, 1:2], lc[h][:], axis=mybir.AxisListType.X,
                                    op=Alu.max)
            avmx.append(am)
        psfc = pp.tile([16, 2], f32, tag="ps", name="psfc")
        for h in range(2):
            nc.tensor.matmul(psfc[:], caw1[:, h, :], avmx[h][:],
                             start=(h == 0), stop=(h == 1))
        fc1 = ev.tile([16, 2], f32r, tag="fc1", name="fc1", bufs=1)
        nc.vector.tensor_scalar(fc1[:], psfc[:], 0.0, None, op0=Alu.max)
        schl = []
        schq = []
        for h in range(2):
            ps2 = pp.tile([128, 2], f32, tag="ps", name="ps2")
            nc.tensor.matmul(ps2[:], caw2[:, 128 * h:128 * h + 128], fc1[:],
                             start=True, stop=True)
            fcs = ev.tile([128, 2], f32, tag=f"fcs{h}", name=f"fcs{h}", bufs=1)
            nc.scalar.copy(fcs[:], ps2[:])
            sc = ev.tile([128, 1], f32, tag=f"sch{h}", name=f"sch{h}", bufs=1)
            nc.vector.tensor_tensor(sc[:], fcs[:, 0:1], fcs[:, 1:2], op=Alu.add)
            nc.scalar.activation(sc[:], sc[:], Act.Sigmoid)
            scl = ev.tile([128, 1], f32, tag=f"schl{h}", name=f"schl{h}", bufs=1)
            nc.vector.tensor_scalar(scl[:], sc[:], 1.0 / L, None, op0=Alu.mult)
            scq = ev.tile([128, 1], f32r, tag=f"schq{h}", name=f"schq{h}", bufs=1)
            nc.vector.tensor_scalar(scq[:], sc[:], 1.0 / 256.0, None, op0=Alu.mult)
            schl.append(scl)
            schq.append(scq)
            schq.append(sc)

        # ---- SA: channel mean (PE) + max (gpsimd all-reduce) --------------
        for j in range(8):
            js = slice(j * 512, (j + 1) * 512)
            psm = pp.tile([1, 512], f32, tag="ps", name="psm")
            for h in range(2):
                nc.tensor.matmul(psm[:], schq[2 * h][:], lc[h][:, js],
                                 start=(h == 0), stop=(h == 1))
            sm = ev.tile([1, 512], f32, tag="sm", name="sm", bufs=2)
            nc.scalar.copy(sm[:], psm[:])
            nc.sync.dma_start(sab_d[0:1, js], sm[:])
            m0 = ev.tile([128, 512], bf16, tag="m0", name="m0", bufs=2)
            nc.vector.tensor_scalar(m0[:], lc[0][:, js], schq[1][:], None,
                                    op0=Alu.mult)
            m1 = ev.tile([128, 512], bf16, tag="m1", name="m1", bufs=2)
            nc.vector.tensor_scalar(m1[:], lc[1][:, js], schq[3][:], None,
                                    op0=Alu.mult)
            mh = ev.tile([128, 512], bf16, tag="mh", name="mh", bufs=2)
            nc.vector.tensor_tensor(mh[:], m0[:], m1[:], op=Alu.max)
            pr = ev.tile([128, 512], bf16, tag="pr", name="pr", bufs=2)
            nc.gpsimd.partition_all_reduce(pr[:], mh[:], channels=128,
                                           reduce_op=bass_isa.ReduceOp.max)
            sx = ev.tile([1, 512], f32, tag="sx", name="sx", bufs=2)
            nc.vector.tensor_copy(sx[:], pr[0:1, :])
            nc.sync.dma_start(sab_d[1:2, js], sx[:])
        msh = []
        for kh in range(3):
            mt = ev.tile([64, 2, 66], f32, tag=f"msh{kh}", name=f"msh{kh}", bufs=1)
            nc.vector.memset(mt[:], 0.0)
            sr = sab_d[:].rearrange("c (h w) -> h c w", w=64)
            if kh == 0:
                nc.sync.dma_start(mt[1:64, :, 1:65], sr[0:63])
            elif kh == 1:
                nc.sync.dma_start(mt[0:64, :, 1:65], sr[0:64])
            else:
                nc.sync.dma_start(mt[0:63, :, 1:65], sr[1:64])
            msh.append(mt)
        acc = [ev.tile([64, 64], f32, tag=f"sacc{i}", name=f"sacc{i}", bufs=1)
               for i in range(2)]
        k = 0
        for ch in range(2):
            for kh in range(3):
                for kw in range(3):
                    w = float(sa_w[0, ch, kh, kw])
                    src = msh[kh][0:64, ch, kw:kw + 64]
                    if k == 0:
                        nc.vector.tensor_scalar(acc[0][:], src, w, None,
                                                op0=Alu.mult)
                    else:
                        nc.vector.scalar_tensor_tensor(
                            acc[k % 2][:], src, w, acc[(k + 1) % 2][:],
                            op0=Alu.mult, op1=Alu.add)
                    k += 1
        ssp = ev.tile([64, 64], f32r, tag="ssp", name="ssp", bufs=1)
        nc.scalar.activation(ssp[:], acc[(k + 1) % 2][:], Act.Sigmoid)
        nc.sync.dma_start(ssb_d[:].rearrange("o (h w) -> (o h) w", w=64),
                          ssp[:].bitcast(f32))
        ssprow = ev.tile([1, NPIX], f32r, tag="ssprow", name="ssprow", bufs=1)
        nc.sync.dma_start(ssprow[:], ssb_d[:].bitcast(f32r))

        # ---- lam = lc * (s_ch/L) * s_sp (fp32) ----------------------------
        lam = [[None] * 8 for _ in range(2)]
        for j in range(8):
            psb = pp.tile([128, 512], f32, tag="ps", name="psb")
            nc.tensor.matmul(psb[:], ones1[:], ssprow[:, j * 512:(j + 1) * 512],
                             start=True, stop=True)
            for h in range(2):
                lslice = lc[h][:, j * 512:(j + 1) * 512]
                nc.vector.scalar_tensor_tensor(
                    lslice, lslice, schl[h][:],
                    psb[:], op0=Alu.mult, op1=Alu.mult)
                lam[h][j] = lslice

        # ---- LISTA --------------------------------------------------------
        for j in range(8):
            js = slice(j * 512, (j + 1) * 512)
            zprev = [None, None]
            for t in range(num_iters + 1):
                znew = [None, None]
                for h in range(2):
                    ps = pp.tile([128, 512], f32, tag="ps", name="psv")
                    if t == 0:
                        nc.tensor.matmul(ps[:], dctF[:, 128 * h:128 * h + 128],
                                         xF[:, js], start=True, stop=True)
                        nc.scalar.activation(yLr[h][:, js], ps[:], Act.Copy,
                                             scale=1.0 / L)
                    else:
                        nc.tensor.matmul(ps[:], ident[:], yLr[h][:, js],
                                         start=True, stop=False)
                        nc.tensor.matmul(ps[:], s_t[:, 0, 128 * h:128 * h + 128],
                                         zprev[0][:], start=False, stop=False)
                        nc.tensor.matmul(ps[:], s_t[:, 1, 128 * h:128 * h + 128],
                                         zprev[1][:], start=False, stop=True)
                    z = zp.tile([128, 512], f32r, tag=f"z{h}", name=f"z{h}")
                    nc.vector._custom_dve(SOFT, out=z[:], in0=ps[:],
                                          in1=lam[h][j][:])
                    znew[h] = z
                zprev = znew
            for h in range(2):
                nc.sync.dma_start(zo_d[128 * h:128 * h + 128, js],
                                  zprev[h][:].bitcast(f32))
            psr = pp.tile([64, 512], f32, tag="ps", name="psr")
            for h in range(2):
                nc.tensor.matmul(psr[:], dctt[:, h, :], zprev[h][:],
                                 start=(h == 0), stop=(h == 1))
            rc = ev.tile([64, 512], f32, tag="rc", name="rc", bufs=2)
            nc.scalar.copy(rc[:], psr[:])
            nc.sync.dma_start(ro_d[:, js], rc[:])

        late_cm.__exit__(None, None, None)
        zp_cm.__exit__(None, None, None)
        pp_cm.__exit__(None, None, None)
        ev_cm.__exit__(None, None, None)
        wp_cm.__exit__(None, None, None)

    nc.compile()
    return nc


def _host_prep(inputs):
    b16 = ml_dtypes.bfloat16
    cw = np.ascontiguousarray(inputs["conv_w"], dtype=np.float32)
    cwp = np.zeros((128, 2, 3, 128), np.float32)
    cws = np.zeros((64, 2, 3, 128), np.float32)
    for h in range(2):
        for dc in range(3):
            cwp[0:64, h, dc, :] = cw[128 * h:128 * h + 128, :, 0, dc].T
            cwp[64:128, h, dc, :] = cw[128 * h:128 * h + 128, :, 1, dc].T
            cws[:, h, dc, :] = cw[128 * h:128 * h + 128, :, 2, dc].T
    b2 = np.ascontiguousarray(
        np.asarray(inputs["conv_b"], np.float32).reshape(2, 128).T)

    def res_pack(w1, w2):
        w1 = np.asarray(w1, np.float32)
        r1 = np.zeros((128, 9, 2, 128), np.float32)
        for kh in range(3):
            for kw in range(3):
                for h in range(2):
                    r1[:, kh * 3 + kw, h, :] = w1[:, 128 * h:128 * h + 128, kh, kw].T
        r2 = np.ascontiguousarray(np.asarray(w2, np.float32)[:, :, 0, 0].T)
        return r1.astype(b16), r2.astype(b16)

    rw1a, rw2a = res_pack(inputs["res1_w1"], inputs["res1_w2"])
    rw1b, rw2b = res_pack(inputs["res2_w1"], inputs["res2_w2"])
    caw1 = np.ascontiguousarray(
        np.asarray(inputs["ca_w1"], np.float32).T.reshape(2, 128, 16)
        .transpose(1, 0, 2))
    caw2 = np.ascontiguousarray(np.asarray(inputs["ca_w2"], np.float32).T)
    D = np.asarray(inputs["Dict"], np.float32)
    L = float(np.asarray(inputs["L"]))
    S = np.ascontiguousarray(
        (np.eye(256, dtype=np.float32) - (D.T @ D) / np.float32(L)).T
        .reshape(2, 128, 256).transpose(1, 0, 2))
    dctt = np.ascontiguousarray(D.T.reshape(2, 128, 64).transpose(1, 0, 2))
    return dict(cwp=cwp, cws=cws, b2=b2,
                rw1a=rw1a, rw2a=rw2a, rw1b=rw1b, rw2b=rw2b, caw1=caw1,
                caw2=caw2, S=S, Dct=D, DctT=dctt,
                ident=np.eye(128, dtype=np.float32)), L


def kernel(**inputs):
    num_iters = int(np.asarray(inputs["num_iters"]))
    sa_w = np.asarray(inputs["sa_w"], np.float32)
    weights, L = _host_prep(inputs)
    nc = build_nc(num_iters, L, sa_w)
    x = np.ascontiguousarray(np.asarray(inputs["x"], np.float32))
    in_maps = [dict(weights, x=x[b]) for b in range(8)]
    res = run_bass_kernel_spmd(nc, in_maps, core_ids=list(range(8)))
    z = np.stack([res.results[b]["zo"].reshape(256, 64, 64) for b in range(8)])
    rec = np.stack([res.results[b]["ro"].reshape(64, 64, 64) for b in range(8)])
    return (z, rec, np.asarray(inputs["Dict"], np.float32))


if __name__ == "__main__":
    d = np.load("/root/problem/ref_cache.npz")
    ins = {k: d[k] for k in ["x", "conv_w", "conv_b", "res1_w1", "res1_w2",
                             "res2_w1", "res2_w2", "ca_w1", "ca_w2", "sa_w",
                             "Dict", "L", "num_iters"]}
    out = kernel(**ins)
    for i, name in enumerate(["z", "recon", "Dict"]):
        ref = d[f"out{i}"]
        got = out[i]
        num = np.abs(got - ref).max()
        den = np.abs(ref).max()
        print(f"{name}: absmax diff {num:.3e}  scale {den:.3e}  rel {num/den:.3e}")


# revision 18
# speedup vs baseline: 1.1959x; 1.1959x over previous
"""AttentiveLISTA Trainium2 kernel — pure data parallel over 8 NeuronCores.

Per core (B=1 image, C=64, H=W=64, A=256):
  conv3x3(64->256)+bias -> 2x residual blocks -> CBAM (channel+spatial
  attention) -> per-pixel LISTA (z0 = soft(y); 16x z = soft(z@S + y/L)).

Implementation notes:
  * channels on SBUF partitions, flattened padded 66x66 image on the free dim
    (base offset +2, buffers [*, 4360]); 3x3 convs = shifted matmuls (conv1
    pairs taps kh=0/kh=1 per K=128 chunk via a second +66-shifted x copy).
  * The conv/CBAM tower only produces the soft-thresholds lam (the LISTA data
    path is x/Dict/S alone), so the tower runs in bf16: same 1 cyc/row PE
    speed, half the SBUF, and a ~1e-2-relative lam error that perturbs z by
    ~|dlam| ~ 3e-5 absolute.
  * LISTA matmuls in float32r (~13-bit-mantissa fp32, 1 PE cycle/row).
  * LISTA soft-threshold = one fused custom DVE op per tile:
        out = v > 0 ? relu(v - lam) : -relu(-v - lam)
    reading v straight from PSUM, writing z as f32r (matmul-ready).
  * y/L rides the loop's PSUM accumulation as a 3rd K=64 matmul (Dict/L @ x),
    so no separate add pass exists.
"""
import os
import sys

for _p in ("/opt/trn_rl_repo", "/opt/trn_rl_repo/concourse"):
    if _p not in sys.path:
        sys.path.insert(0, _p)

import numpy as np
import ml_dtypes

import concourse.bass as bass
from concourse import bacc, mybir, bass_isa
import concourse.tile as tile
from concourse.bass_utils import run_bass_kernel_spmd

f32 = mybir.dt.float32
f32r = mybir.dt.float32r
bf16 = mybir.dt.bfloat16
Alu = mybir.AluOpType
Act = mybir.ActivationFunctionType

W_PAD = 66           # padded row width
BASE = 2             # flat offset of padded pixel 0 inside the [*, 4360] bufs
BUFW = 4360
INT0 = BASE + W_PAD  # 68: first interior-row flat position
INTN = 64 * W_PAD    # 4224: contiguous interior span (rows 1..64 incl. side pads)
NPIX = 4096

_SOFT_OP = None


def _register_softshrink():
    global _SOFT_OP
    if _SOFT_OP is not None:
        return _SOFT_OP
    import concourse.dve_ops as dve_ops
    from concourse.dve_spec import Spec, Src0, Src1, Zero, relu, select, lower
    from concourse.dve_uop import DveOpSpec

    name = "SOFTSHRINK_ANT"
    for op in dve_ops.OPS:
        if op.name == name:
            _SOFT_OP = op
            return op

    def _reference(in0, in1, s0, s1, imm2):
        in0 = in0.astype(np.float32)
        return np.where(in0 > 0, np.maximum(in0 - in1, 0.0),
                        -np.maximum(-in0 - in1, 0.0)).astype(np.float32)

    body = select(Src0 > Zero, relu(Src0 - Src1),
                  Zero - relu(Zero - Src0 - Src1))
    spec = Spec(body=body, reference=_reference)
    row = dve_ops._CUSTOM_DVE_ROW_BASE + len(dve_ops.OPS)
    assert row < 0x20
    shas = {}
    for ver in ("v3", "v4"):
        tmp = DveOpSpec(name=name, opcode=row, uops=lower(spec, ver=ver),
                        rd1_en=True)
        shas[ver] = tmp.sha(ver)
    op = dve_ops.DveOp(name, spec, subdim=False, uops_sha=shas)
    dve_ops.OPS.append(op)
    dve_ops.CUSTOM_DVE_SPECS[name] = spec
    dve_ops._SUB_OPCODE_FOR_NAME[name] = row
    _SOFT_OP = op
    return op


def _chunks512(start, span):
    out = []
    o = start
    while o < start + span:
        n = min(512, start + span - o)
        out.append((o, n))
        o += n
    return out


def build_nc(num_iters, L, sa_w):
    SOFT = _register_softshrink()
    nc = bacc.Bacc("TRN2", target_bir_lowering=False, debug=False)

    x_d = nc.dram_tensor("x", [64, 64, 64], f32, kind="ExternalInput")
    cwp_d = nc.dram_tensor("cwp", [128, 2, 3, 128], f32, kind="ExternalInput")
    cws_d = nc.dram_tensor("cws", [64, 2, 3, 128], f32, kind="ExternalInput")
    b2_d = nc.dram_tensor("b2", [128, 2], f32, kind="ExternalInput")
    rw1a_d = nc.dram_tensor("rw1a", [128, 9, 2, 128], bf16, kind="ExternalInput")
    rw2a_d = nc.dram_tensor("rw2a", [128, 256], bf16, kind="ExternalInput")
    rw1b_d = nc.dram_tensor("rw1b", [128, 9, 2, 128], bf16, kind="ExternalInput")
    rw2b_d = nc.dram_tensor("rw2b", [128, 256], bf16, kind="ExternalInput")
    caw1_d = nc.dram_tensor("caw1", [128, 2, 16], f32, kind="ExternalInput")
    caw2_d = nc.dram_tensor("caw2", [16, 256], f32, kind="ExternalInput")
    s_d = nc.dram_tensor("S", [128, 2, 256], f32, kind="ExternalInput")
    dict_d = nc.dram_tensor("Dct", [64, 256], f32, kind="ExternalInput")
    ident_d = nc.dram_tensor("ident", [128, 128], f32, kind="ExternalInput")
    dictt_d = nc.dram_tensor("DctT", [128, 2, 64], f32, kind="ExternalInput")

    zo_d = nc.dram_tensor("zo", [256, NPIX], f32, kind="ExternalOutput")
    ro_d = nc.dram_tensor("ro", [64, NPIX], f32, kind="ExternalOutput")
    sab_d = nc.dram_tensor("sab", [2, 4096], f32)
    ssb_d = nc.dram_tensor("ssb", [1, 4096], f32)

    conv_chunks = _chunks512(INT0, INTN)

    with tile.TileContext(nc) as tc:
        wp_cm = tc.tile_pool(name="wp", bufs=1)
        ev_cm = tc.tile_pool(name="ev", bufs=4)
        pp_cm = tc.tile_pool(name="pp", bufs=8, space="PSUM")
        late_cm = tc.tile_pool(name="late", bufs=1)
        cw_cm = tc.tile_pool(name="cw", bufs=1)
        t2_cm = tc.tile_pool(name="t2", bufs=1)
        t1_cm = tc.tile_pool(name="t1", bufs=1)
        x2_cm = tc.tile_pool(name="x2p", bufs=1)
        wp = wp_cm.__enter__()
        ev = ev_cm.__enter__()
        pp = pp_cm.__enter__()
        late = late_cm.__enter__()
        cw = cw_cm.__enter__()
        t2 = t2_cm.__enter__()
        t1 = t1_cm.__enter__()

        # ---- long-lived small weights ------------------------------------
        b2 = wp.tile([128, 2], f32, name="b2")
        nc.sync.dma_start(b2[:], b2_d[:])
        caw1 = wp.tile([128, 2, 16], f32r, name="caw1")
        nc.sync.dma_start(caw1[:], caw1_d[:].bitcast(f32r))
        caw2 = wp.tile([16, 256], f32r, name="caw2")
        nc.sync.dma_start(caw2[:], caw2_d[:].bitcast(f32r))
        s_t = wp.tile([128, 2, 256], f32r, name="s_t")
        nc.sync.dma_start(s_t[:], s_d[:].bitcast(f32r))
        dctF = wp.tile([64, 256], f32, name="dctF")
        nc.sync.dma_start(dctF[:], dict_d[:])
        ident = wp.tile([128, 128], f32r, name="ident")
        nc.sync.dma_start(ident[:], ident_d[:].bitcast(f32r))
        dctt = wp.tile([128, 2, 64], f32r, name="dctt")
        nc.sync.dma_start(dctt[:], dictt_d[:].bitcast(f32r))
        ones1 = wp.tile([1, 128], f32r, name="ones1")
        nc.vector.memset(ones1.bitcast(f32)[:], 1.0)
        xF = wp.tile([64, NPIX], f32, name="xF")
        nc.sync.dma_start(xF[:], x_d[:].rearrange("c h w -> c (h w)"))

        # lam-path carrier: l -> l1 -> l2, updated in place (compact f32r)
        lc = [late.tile([128, NPIX], f32r, name=f"lc{h}") for h in range(2)]
        yLr = [late.tile([128, NPIX], f32r, name=f"yLr{h}") for h in range(2)]

        # ---- conv1 weights (f32r) + res weights (bf16) --------------------
        cwp = cw.tile([128, 2, 3, 128], f32r, name="cwp")
        nc.sync.dma_start(cwp[:], cwp_d[:].bitcast(f32r))
        cws = cw.tile([64, 2, 3, 128], f32r, name="cws")
        nc.sync.dma_start(cws[:], cws_d[:].bitcast(f32r))
        rw1 = cw.tile([128, 9, 2, 128], bf16, tag="rw1", name="rw1")
        nc.sync.dma_start(rw1[:], rw1a_d[:])
        rw2 = cw.tile([128, 256], bf16, tag="rw2", name="rw2")
        nc.sync.dma_start(rw2[:], rw2a_d[:])

        x2p = x2_cm.__enter__()
        x2 = x2p.tile([128, BUFW], f32r, name="x2")
        nc.vector.memset(x2.bitcast(f32)[0:64, :], 0.0)
        nc.gpsimd.memset(x2.bitcast(f32)[64:128, :], 0.0)
        dst0 = x2[0:64, INT0:INT0 + INTN].rearrange("p (r c) -> p r c", c=W_PAD)[:, :, 1:65]
        nc.sync.dma_start(dst0, x_d[:].bitcast(f32r))
        dst1 = x2[64:128, BASE:BASE + INTN].rearrange("p (r c) -> p r c", c=W_PAD)[:, :, 1:65]
        nc.sync.dma_start(dst1, x_d[:].bitcast(f32r))

        def win3d(buf, j, d=0):
            base = INT0 + j * 8 * W_PAD + d
            return buf[:, base:base + 8 * W_PAD] \
                .rearrange("p (r c) -> p r c", c=W_PAD)[:, :, 1:65]

        def win3d64(buf, j, parts=128):
            base = INT0 + j * 8 * W_PAD
            return buf[0:parts, base:base + 8 * W_PAD] \
                .rearrange("p (r c) -> p r c", c=W_PAD)[:, :, 1:65]

        # ---- conv1 (row-aligned): lc = W@x + b (f32r), rl (bf16 padded) ---
        rl_t = [t1.tile([128, BUFW], bf16, name=f"rlA{h}") for h in range(2)]
        for h in range(2):
            for j in range(8):
                ps = pp.tile([128, 512], f32, tag="ps", name="ps")
                for dc in range(3):
                    nc.tensor.matmul(ps[:], cwp[:, h, dc, :],
                                     win3d(x2, j, dc - 67),
                                     start=(dc == 0), stop=False)
                for dc in range(3):
                    base = INT0 + j * 8 * W_PAD + 65 + dc
                    rhs = x2[0:64, base:base + 8 * W_PAD] \
                        .rearrange("p (r c) -> p r c", c=W_PAD)[:, :, 1:65]
                    nc.tensor.matmul(ps[:], cws[:, h, dc, :], rhs,
                                     start=False, stop=(dc == 2))
                nc.scalar.activation(lc[h][:, j * 512:(j + 1) * 512], ps[:],
                                     Act.Identity, bias=b2[:, h:h + 1])
                nc.vector.tensor_scalar(
                    win3d(rl_t[h], j), ps[:].rearrange("p (r c) -> p r c", c=64),
                    b2[:, h:h + 1], 0.0, op0=Alu.add, op1=Alu.max)
        for h in range(2):
            nc.gpsimd.memset(rl_t[h][:, 0:INT0], 0.0)
            nc.gpsimd.memset(rl_t[h][:, INT0 + INTN:BUFW], 0.0)
            side = rl_t[h][:, INT0:INT0 + INTN].rearrange("p (r c) -> p r c", c=W_PAD)
            nc.gpsimd.memset(side[:, :, 0:1], 0.0)
            nc.gpsimd.memset(side[:, :, 65:66], 0.0)
        x2_cm.__exit__(None, None, None)

        def conv3x3(rin, rw, tname):
            rt = t2.tile([128, BUFW], bf16, tag="rt", name=tname)
            for ci, (off, n) in enumerate(conv_chunks):
                ps = pp.tile([128, 512], f32, tag="ps", name="ps")
                k = 0
                for kh in range(3):
                    for kw in range(3):
                        d = (kh - 1) * W_PAD + (kw - 1)
                        for h in range(2):
                            nc.tensor.matmul(ps[:, :n], rw[:, kh * 3 + kw, h, :],
                                             rin[h][:, off + d: off + d + n],
                                             start=(k == 0), stop=(k == 17))
                            k += 1
                if ci % 2 == 0:
                    nc.vector.tensor_scalar(rt[:, off:off + n], ps[:, :n],
                                            0.0, None, op0=Alu.max)
                else:
                    nc.scalar.activation(rt[:, off:off + n], ps[:, :n], Act.Relu)
            return rt

        # ---- res1: lc += conv1x1(rt1); rl1 = relu(lc) ---------------------
        rt1 = conv3x3(rl_t, rw1, "rt1")
        rl1 = [t2.tile([128, BUFW], bf16, name=f"rlB{h}") for h in range(2)]
        for h in range(2):
            for j in range(8):
                ps = pp.tile([128, 512], f32, tag="ps", name="ps")
                nc.tensor.matmul(ps[:], rw2[:, 128 * h:128 * h + 128],
                                 win3d64(rt1, j), start=True, stop=True)
                nc.vector.tensor_tensor(lc[h][:, j * 512:(j + 1) * 512], ps[:],
                                        lc[h][:, j * 512:(j + 1) * 512],
                                        op=Alu.add)
                nc.scalar.activation(
                    win3d(rl1[h], j),
                    lc[h][:, j * 512:(j + 1) * 512].rearrange("p (r c) -> p r c", c=64),
                    Act.Relu)
            nc.gpsimd.memset(rl1[h][:, 0:INT0], 0.0)
            nc.gpsimd.memset(rl1[h][:, INT0 + INTN:BUFW], 0.0)
            side = rl1[h][:, INT0:INT0 + INTN].rearrange("p (r c) -> p r c", c=W_PAD)
            nc.gpsimd.memset(side[:, :, 0:1], 0.0)
            nc.gpsimd.memset(side[:, :, 65:66], 0.0)
        t1_cm.__exit__(None, None, None)

        # res2 weights reuse the res1 slots
        rw1b = cw.tile([128, 9, 2, 128], bf16, tag="rw1", name="rw1b")
        nc.sync.dma_start(rw1b[:], rw1b_d[:])
        rw2b = cw.tile([128, 256], bf16, tag="rw2", name="rw2b")
        nc.sync.dma_start(rw2b[:], rw2b_d[:])

        # ---- res2: lc += conv1x1(rt2), channel sums ride the update -------
        rt2 = conv3x3(rl1, rw1b, "rt2")
        sums = [ev.tile([128, 8], f32, tag=f"sum{h}", name=f"sum{h}", bufs=1)
                for h in range(2)]
        for h in range(2):
            for j in range(8):
                ps = pp.tile([128, 512], f32, tag="ps", name="ps")
                nc.tensor.matmul(ps[:], rw2b[:, 128 * h:128 * h + 128],
                                 win3d64(rt2, j), start=True, stop=True)
                nc.vector.scalar_tensor_tensor(
                    lc[h][:, j * 512:(j + 1) * 512], ps[:], 0.0,
                    lc[h][:, j * 512:(j + 1) * 512], op0=Alu.add, op1=Alu.add,
                    accum_out=sums[h][:, j:j + 1])
        t2_cm.__exit__(None, None, None)
        cw_cm.__exit__(None, None, None)
        zp_cm = tc.tile_pool(name="zp", bufs=2)
        zp = zp_cm.__enter__()

        # ---- CBAM channel attention --------------------------------------
        avmx = []
        for h in range(2):
            ssum = ev.tile([128, 1], f32, tag="ssum", name="ssum", bufs=2)
            nc.vector.tensor_reduce(ssum[:], sums[h][:], axis=mybir.AxisListType.X,
                                    op=Alu.add)
            am = ev.tile([128, 2], f32r, tag=f"avmx{h}", name=f"avmx{h}", bufs=1)
            nc.vector.tensor_scalar(am[:, 0:1], ssum[:], 1.0 / NPIX, None,
                                    op0=Alu.mult)
            nc.vector.tensor_reduce(am[:, 1:2], lc[h][:], axis=mybir.AxisListType.X,
                                    op=Alu.max)
            avmx.append(am)
        psfc = pp.tile([16, 2], f32, tag="ps", name="psfc")
        for h in range(2):
            nc.tensor.matmul(psfc[:], caw1[:, h, :], avmx[h][:],
                             start=(h == 0), stop=(h == 1))
        fc1 = ev.tile([16, 2], f32r, tag="fc1", name="fc1", bufs=1)
        nc.vector.tensor_scalar(fc1[:], psfc[:], 0.0, None, op0=Alu.max)
        schl = []
        schq = []
        for h in range(2):
            ps2 = pp.tile([128, 2], f32, tag="ps", name="ps2")
            nc.tensor.matmul(ps2[:], caw2[:, 128 * h:128 * h + 128], fc1[:],
                             start=True, stop=True)
            fcs = ev.tile([128, 2], f32, tag=f"fcs{h}", name=f"fcs{h}", bufs=1)
            nc.scalar.copy(fcs[:], ps2[:])
            sc = ev.tile([128, 1], f32, tag=f"sch{h}", name=f"sch{h}", bufs=1)
            nc.vector.tensor_tensor(sc[:], fcs[:, 0:1], fcs[:, 1:2], op=Alu.add)
            nc.scalar.activation(sc[:], sc[:], Act.Sigmoid)
            scl = ev.tile([128, 1], f32, tag=f"schl{h}", name=f"schl{h}", bufs=1)
            nc.vector.tensor_scalar(scl[:], sc[:], 1.0 / L, None, op0=Alu.mult)
            scq = ev.tile([128, 1], f32r, tag=f"schq{h}", name=f"schq{h}", bufs=1)
            nc.vector.tensor_scalar(scq[:], sc[:], 1.0 / 256.0, None, op0=Alu.mult)
            schl.append(scl)
            schq.append(scq)
            schq.append(sc)

        # ---- SA: channel mean (PE) + max (gpsimd all-reduce) --------------
        for j in range(8):
            js = slice(j * 512, (j + 1) * 512)
            psm = pp.tile([1, 512], f32, tag="ps", name="psm")
            for h in range(2):
                nc.tensor.matmul(psm[:], schq[2 * h][:], lc[h][:, js],
                                 start=(h == 0), stop=(h == 1))
            sm = ev.tile([1, 512], f32, tag="sm", name="sm", bufs=2)
            nc.scalar.copy(sm[:], psm[:])
            nc.sync.dma_start(sab_d[0:1, js], sm[:])
            m0 = ev.tile([128, 512], bf16, tag="m0", name="m0", bufs=2)
            nc.vector.tensor_scalar(m0[:], lc[0][:, js], schq[1][:], None,
                                    op0=Alu.mult)
            m1 = ev.tile([128, 512], bf16, tag="m1", name="m1", bufs=2)
            nc.vector.tensor_scalar(m1[:], lc[1][:, js], schq[3][:], None,
                                    op0=Alu.mult)
            mh = ev.tile([128, 512], bf16, tag="mh", name="mh", bufs=2)
            nc.vector.tensor_tensor(mh[:], m0[:], m1[:], op=Alu.max)
            pr = ev.tile([128, 512], bf16, tag="pr", name="pr", bufs=2)
            nc.gpsimd.partition_all_reduce(pr[:], mh[:], channels=128,
                                           reduce_op=bass_isa.ReduceOp.max)
            sx = ev.tile([1, 512], f32, tag="sx", name="sx", bufs=2)
            nc.vector.tensor_copy(sx[:], pr[0:1, :])
            nc.sync.dma_start(sab_d[1:2, js], sx[:])
        msh = []
        for kh in range(3):
            mt = ev.tile([64, 2, 66], f32, tag=f"msh{kh}", name=f"msh{kh}", bufs=1)
            nc.vector.memset(mt[:], 0.0)
            sr = sab_d[:].rearrange("c (h w) -> h c w", w=64)
            if kh == 0:
                nc.sync.dma_start(mt[1:64, :, 1:65], sr[0:63])
            elif kh == 1:
                nc.sync.dma_start(mt[0:64, :, 1:65], sr[0:64])
            else:
                nc.sync.dma_start(mt[0:63, :, 1:65], sr[1:64])
            msh.append(mt)
        acc = [ev.tile([64, 64], f32, tag=f"sacc{i}", name=f"sacc{i}", bufs=1)
               for i in range(2)]
        k = 0
        for ch in range(2):
            for kh in range(3):
                for kw in range(3):
                    w = float(sa_w[0, ch, kh, kw])
                    src = msh[kh][0:64, ch, kw:kw + 64]
                    if k == 0:
                        nc.vector.tensor_scalar(acc[0][:], src, w, None,
                                                op0=Alu.mult)
                    else:
                        nc.vector.scalar_tensor_tensor(
                            acc[k % 2][:], src, w, acc[(k + 1) % 2][:],
                            op0=Alu.mult, op1=Alu.add)
                    k += 1
        ssp = ev.tile([64, 64], f32r, tag="ssp", name="ssp", bufs=1)
        nc.scalar.activation(ssp[:], acc[(k + 1) % 2][:], Act.Sigmoid)
        nc.sync.dma_start(ssb_d[:].rearrange("o (h w) -> (o h) w", w=64),
                          ssp[:].bitcast(f32))
        ssprow = ev.tile([1, NPIX], f32r, tag="ssprow", name="ssprow", bufs=1)
        nc.sync.dma_start(ssprow[:], ssb_d[:].bitcast(f32r))

        # ---- lam = lc * (s_ch/L) * s_sp (fp32) ----------------------------
        lam = [[None] * 8 for _ in range(2)]
        for j in range(8):
            psb = pp.tile([128, 512], f32, tag="ps", name="psb")
            nc.tensor.matmul(psb[:], ones1[:], ssprow[:, j * 512:(j + 1) * 512],
                             start=True, stop=True)
            for h in range(2):
                lslice = lc[h][:, j * 512:(j + 1) * 512]
                nc.vector.scalar_tensor_tensor(
                    lslice, lslice, schl[h][:],
                    psb[:], op0=Alu.mult, op1=Alu.mult)
                lam[h][j] = lslice

        # ---- LISTA: groups of 4 chunks, t-major round-robin ---------------
        for g in range(2):
            jset = range(4 * g, 4 * g + 4)
            zprev = {j: [None, None] for j in jset}
            for t in range(num_iters + 1):
                for j in jset:
                    js = slice(j * 512, (j + 1) * 512)
                    znew = [None, None]
                    for h in range(2):
                        ps = pp.tile([128, 512], f32, tag="ps", name="psv")
                        if t == 0:
                            nc.tensor.matmul(ps[:], dctF[:, 128 * h:128 * h + 128],
                                             xF[:, js], start=True, stop=True)
                            nc.scalar.activation(yLr[h][:, js], ps[:], Act.Copy,
                                                 scale=1.0 / L)
                        else:
                            nc.tensor.matmul(ps[:], ident[:], yLr[h][:, js],
                                             start=True, stop=False)
                            nc.tensor.matmul(ps[:], s_t[:, 0, 128 * h:128 * h + 128],
                                             zprev[j][0][:], start=False, stop=False)
                            nc.tensor.matmul(ps[:], s_t[:, 1, 128 * h:128 * h + 128],
                                             zprev[j][1][:], start=False, stop=True)
                        z = zp.tile([128, 512], f32r, tag=f"z{j % 4}_{h}",
                                    name=f"z{j % 4}_{h}")
                        nc.vector._custom_dve(SOFT, out=z[:], in0=ps[:],
                                              in1=lam[h][j][:])
                        znew[h] = z
                    zprev[j] = znew
            for j in jset:
                js = slice(j * 512, (j + 1) * 512)
                for h in range(2):
                    nc.sync.dma_start(zo_d[128 * h:128 * h + 128, js],
                                      zprev[j][h][:].bitcast(f32))
                psr = pp.tile([64, 512], f32, tag="ps", name="psr")
                for h in range(2):
                    nc.tensor.matmul(psr[:], dctt[:, h, :], zprev[j][h][:],
                                     start=(h == 0), stop=(h == 1))
                rc = ev.tile([64, 512], f32, tag="rc", name="rc", bufs=2)
                nc.scalar.copy(rc[:], psr[:])
                nc.sync.dma_start(ro_d[:, js], rc[:])

        zp_cm.__exit__(None, None, None)
        late_cm.__exit__(None, None, None)
        pp_cm.__exit__(None, None, None)
        ev_cm.__exit__(None, None, None)
        wp_cm.__exit__(None, None, None)

    nc.compile()
    return nc


def _host_prep(inputs):
    b16 = ml_dtypes.bfloat16
    cw = np.ascontiguousarray(inputs["conv_w"], dtype=np.float32)
    cwp = np.zeros((128, 2, 3, 128), np.float32)
    cws = np.zeros((64, 2, 3, 128), np.float32)
    for h in range(2):
        for dc in range(3):
            cwp[0:64, h, dc, :] = cw[128 * h:128 * h + 128, :, 0, dc].T
            cwp[64:128, h, dc, :] = cw[128 * h:128 * h + 128, :, 1, dc].T
            cws[:, h, dc, :] = cw[128 * h:128 * h + 128, :, 2, dc].T
    b2 = np.ascontiguousarray(
        np.asarray(inputs["conv_b"], np.float32).reshape(2, 128).T)

    def res_pack(w1, w2):
        w1 = np.asarray(w1, np.float32)
        r1 = np.zeros((128, 9, 2, 128), np.float32)
        for kh in range(3):
            for kw in range(3):
                for h in range(2):
                    r1[:, kh * 3 + kw, h, :] = w1[:, 128 * h:128 * h + 128, kh, kw].T
        r2 = np.ascontiguousarray(np.asarray(w2, np.float32)[:, :, 0, 0].T)
        return r1.astype(b16), r2.astype(b16)

    rw1a, rw2a = res_pack(inputs["res1_w1"], inputs["res1_w2"])
    rw1b, rw2b = res_pack(inputs["res2_w1"], inputs["res2_w2"])
    caw1 = np.ascontiguousarray(
        np.asarray(inputs["ca_w1"], np.float32).T.reshape(2, 128, 16)
        .transpose(1, 0, 2))
    caw2 = np.ascontiguousarray(np.asarray(inputs["ca_w2"], np.float32).T)
    D = np.asarray(inputs["Dict"], np.float32)
    L = float(np.asarray(inputs["L"]))
    S = np.ascontiguousarray(
        (np.eye(256, dtype=np.float32) - (D.T @ D) / np.float32(L)).T
        .reshape(2, 128, 256).transpose(1, 0, 2))
    dctt = np.ascontiguousarray(D.T.reshape(2, 128, 64).transpose(1, 0, 2))
    return dict(cwp=cwp, cws=cws, b2=b2,
                rw1a=rw1a, rw2a=rw2a, rw1b=rw1b, rw2b=rw2b, caw1=caw1,
                caw2=caw2, S=S, Dct=D, DctT=dctt,
                ident=np.eye(128, dtype=np.float32)), L


def kernel(**inputs):
    num_iters = int(np.asarray(inputs["num_iters"]))
    sa_w = np.asarray(inputs["sa_w"], np.float32)
    weights, L = _host_prep(inputs)
    nc = build_nc(num_iters, L, sa_w)
    x = np.ascontiguousarray(np.asarray(inputs["x"], np.float32))
    in_maps = [dict(weights, x=x[b]) for b in range(8)]
    res = run_bass_kernel_spmd(nc, in_maps, core_ids=list(range(8)))
    z = np.stack([res.results[b]["zo"].reshape(256, 64, 64) for b in range(8)])
    rec = np.stack([res.results[b]["ro"].reshape(64, 64, 64) for b in range(8)])
    return (z, rec, np.asarray(inputs["Dict"], np.float32))


if __name__ == "__main__":
    d = np.load("/root/problem/ref_cache.npz")
    ins = {k: d[k] for k in ["x", "conv_w", "conv_b", "res1_w1", "res1_w2",
                             "res2_w1", "res2_w2", "ca_w1", "ca_w2", "sa_w",
                             "Dict", "L", "num_iters"]}
    out = kernel(**ins)
    for i, name in enumerate(["z", "recon", "Dict"]):
        ref = d[f"out{i}"]
        got = out[i]
        num = np.abs(got - ref).max()
        den = np.abs(ref).max()
        print(f"{name}: absmax diff {num:.3e}  scale {den:.3e}  rel {num/den:.3e}")


# revision 19
# speedup vs baseline: 1.2453x; 1.0414x over previous
"""AttentiveLISTA Trainium2 kernel — pure data parallel over 8 NeuronCores.

Per core (B=1 image, C=64, H=W=64, A=256):
  conv3x3(64->256)+bias -> 2x residual blocks -> CBAM (channel+spatial
  attention) -> per-pixel LISTA (z0 = soft(y); 16x z = soft(z@S + y/L)).

Implementation notes:
  * channels on SBUF partitions, flattened padded 66x66 image on the free dim
    (base offset +2, buffers [*, 4360]); 3x3 convs = shifted matmuls (conv1
    pairs taps kh=0/kh=1 per K=128 chunk via a second +66-shifted x copy).
  * The conv/CBAM tower only produces the soft-thresholds lam (the LISTA data
    path is x/Dict/S alone), so the tower runs in bf16: same 1 cyc/row PE
    speed, half the SBUF, and a ~1e-2-relative lam error that perturbs z by
    ~|dlam| ~ 3e-5 absolute.
  * LISTA matmuls in float32r (~13-bit-mantissa fp32, 1 PE cycle/row).
  * LISTA soft-threshold = one fused custom DVE op per tile:
        out = v > 0 ? relu(v - lam) : -relu(-v - lam)
    reading v straight from PSUM, writing z as f32r (matmul-ready).
  * y/L rides the loop's PSUM accumulation as a 3rd K=64 matmul (Dict/L @ x),
    so no separate add pass exists.
"""
import os
import sys

for _p in ("/opt/trn_rl_repo", "/opt/trn_rl_repo/concourse"):
    if _p not in sys.path:
        sys.path.insert(0, _p)

import numpy as np
import ml_dtypes

import concourse.bass as bass
from concourse import bacc, mybir, bass_isa
import concourse.tile as tile
from concourse.bass_utils import run_bass_kernel_spmd

f32 = mybir.dt.float32
f32r = mybir.dt.float32r
bf16 = mybir.dt.bfloat16
Alu = mybir.AluOpType
Act = mybir.ActivationFunctionType

W_PAD = 66           # padded row width
BASE = 2             # flat offset of padded pixel 0 inside the [*, 4360] bufs
BUFW = 4360
INT0 = BASE + W_PAD  # 68: first interior-row flat position
INTN = 64 * W_PAD    # 4224: contiguous interior span (rows 1..64 incl. side pads)
NPIX = 4096

_SOFT_OP = None


def _register_softshrink():
    global _SOFT_OP
    if _SOFT_OP is not None:
        return _SOFT_OP
    import concourse.dve_ops as dve_ops
    from concourse.dve_spec import Spec, Src0, Src1, Zero, relu, select, lower
    from concourse.dve_uop import DveOpSpec

    name = "SOFTSHRINK_ANT"
    for op in dve_ops.OPS:
        if op.name == name:
            _SOFT_OP = op
            return op

    def _reference(in0, in1, s0, s1, imm2):
        in0 = in0.astype(np.float32)
        return np.where(in0 > 0, np.maximum(in0 - in1, 0.0),
                        -np.maximum(-in0 - in1, 0.0)).astype(np.float32)

    body = select(Src0 > Zero, relu(Src0 - Src1),
                  Zero - relu(Zero - Src0 - Src1))
    spec = Spec(body=body, reference=_reference)
    row = dve_ops._CUSTOM_DVE_ROW_BASE + len(dve_ops.OPS)
    assert row < 0x20
    shas = {}
    for ver in ("v3", "v4"):
        tmp = DveOpSpec(name=name, opcode=row, uops=lower(spec, ver=ver),
                        rd1_en=True)
        shas[ver] = tmp.sha(ver)
    op = dve_ops.DveOp(name, spec, subdim=False, uops_sha=shas)
    dve_ops.OPS.append(op)
    dve_ops.CUSTOM_DVE_SPECS[name] = spec
    dve_ops._SUB_OPCODE_FOR_NAME[name] = row
    _SOFT_OP = op
    return op


def _chunks512(start, span):
    out = []
    o = start
    while o < start + span:
        n = min(512, start + span - o)
        out.append((o, n))
        o += n
    return out


def build_nc(num_iters, L, sa_w):
    SOFT = _register_softshrink()
    nc = bacc.Bacc("TRN2", target_bir_lowering=False, debug=False)

    x_d = nc.dram_tensor("x", [64, 64, 64], f32, kind="ExternalInput")
    cwp_d = nc.dram_tensor("cwp", [128, 2, 3, 128], f32, kind="ExternalInput")
    cws_d = nc.dram_tensor("cws", [64, 2, 3, 128], f32, kind="ExternalInput")
    b2_d = nc.dram_tensor("b2", [128, 2], f32, kind="ExternalInput")
    rw1a_d = nc.dram_tensor("rw1a", [128, 9, 2, 128], bf16, kind="ExternalInput")
    rw2a_d = nc.dram_tensor("rw2a", [128, 256], bf16, kind="ExternalInput")
    rw1b_d = nc.dram_tensor("rw1b", [128, 9, 2, 128], bf16, kind="ExternalInput")
    rw2b_d = nc.dram_tensor("rw2b", [128, 256], bf16, kind="ExternalInput")
    caw1_d = nc.dram_tensor("caw1", [128, 2, 16], f32, kind="ExternalInput")
    caw2_d = nc.dram_tensor("caw2", [16, 256], f32, kind="ExternalInput")
    s_d = nc.dram_tensor("S", [128, 2, 256], f32, kind="ExternalInput")
    dict_d = nc.dram_tensor("Dct", [64, 256], f32, kind="ExternalInput")
    ident_d = nc.dram_tensor("ident", [128, 128], f32, kind="ExternalInput")
    dictt_d = nc.dram_tensor("DctT", [128, 2, 64], f32, kind="ExternalInput")

    zo_d = nc.dram_tensor("zo", [256, NPIX], f32, kind="ExternalOutput")
    ro_d = nc.dram_tensor("ro", [64, NPIX], f32, kind="ExternalOutput")
    sab_d = nc.dram_tensor("sab", [2, 4096], f32)
    ssb_d = nc.dram_tensor("ssb", [1, 4096], f32)

    conv_chunks = _chunks512(INT0, INTN)

    with tile.TileContext(nc) as tc:
        wp_cm = tc.tile_pool(name="wp", bufs=1)
        ev_cm = tc.tile_pool(name="ev", bufs=4)
        pp_cm = tc.tile_pool(name="pp", bufs=8, space="PSUM")
        late_cm = tc.tile_pool(name="late", bufs=1)
        cw_cm = tc.tile_pool(name="cw", bufs=1)
        t2_cm = tc.tile_pool(name="t2", bufs=1)
        t1_cm = tc.tile_pool(name="t1", bufs=1)
        x2_cm = tc.tile_pool(name="x2p", bufs=1)
        wp = wp_cm.__enter__()
        ev = ev_cm.__enter__()
        pp = pp_cm.__enter__()
        late = late_cm.__enter__()
        cw = cw_cm.__enter__()
        t2 = t2_cm.__enter__()
        t1 = t1_cm.__enter__()

        # ---- long-lived small weights ------------------------------------
        b2 = wp.tile([128, 2], f32, name="b2")
        nc.sync.dma_start(b2[:], b2_d[:])
        caw1 = wp.tile([128, 2, 16], f32r, name="caw1")
        nc.sync.dma_start(caw1[:], caw1_d[:].bitcast(f32r))
        caw2 = wp.tile([16, 256], f32r, name="caw2")
        nc.sync.dma_start(caw2[:], caw2_d[:].bitcast(f32r))
        s_t = wp.tile([128, 2, 256], f32r, name="s_t")
        nc.sync.dma_start(s_t[:], s_d[:].bitcast(f32r))
        dctF = wp.tile([64, 256], f32, name="dctF")
        nc.sync.dma_start(dctF[:], dict_d[:])
        ident = wp.tile([128, 128], f32r, name="ident")
        nc.sync.dma_start(ident[:], ident_d[:].bitcast(f32r))
        dctt = wp.tile([128, 2, 64], f32r, name="dctt")
        nc.sync.dma_start(dctt[:], dictt_d[:].bitcast(f32r))
        ones1 = wp.tile([1, 128], f32r, name="ones1")
        nc.vector.memset(ones1.bitcast(f32)[:], 1.0)
        xF = wp.tile([64, NPIX], f32, name="xF")
        nc.sync.dma_start(xF[:], x_d[:].rearrange("c h w -> c (h w)"))

        # lam-path carrier: l -> l1 -> l2, updated in place (compact f32r)
        lc = [late.tile([128, NPIX], f32r, name=f"lc{h}") for h in range(2)]
        yLr = [late.tile([128, NPIX], f32r, name=f"yLr{h}") for h in range(2)]

        # ---- conv1 weights (f32r) + res weights (bf16) --------------------
        cwp = cw.tile([128, 2, 3, 128], f32r, name="cwp")
        nc.sync.dma_start(cwp[:], cwp_d[:].bitcast(f32r))
        cws = cw.tile([64, 2, 3, 128], f32r, name="cws")
        nc.sync.dma_start(cws[:], cws_d[:].bitcast(f32r))
        rw1 = cw.tile([128, 9, 2, 128], bf16, tag="rw1", name="rw1")
        nc.sync.dma_start(rw1[:], rw1a_d[:])
        rw2 = cw.tile([128, 256], bf16, tag="rw2", name="rw2")
        nc.sync.dma_start(rw2[:], rw2a_d[:])

        x2p = x2_cm.__enter__()
        x2 = x2p.tile([128, BUFW], f32r, name="x2")
        nc.vector.memset(x2.bitcast(f32)[0:64, :], 0.0)
        nc.gpsimd.memset(x2.bitcast(f32)[64:128, :], 0.0)
        dst0 = x2[0:64, INT0:INT0 + INTN].rearrange("p (r c) -> p r c", c=W_PAD)[:, :, 1:65]
        nc.sync.dma_start(dst0, x_d[:].bitcast(f32r))
        dst1 = x2[64:128, BASE:BASE + INTN].rearrange("p (r c) -> p r c", c=W_PAD)[:, :, 1:65]
        nc.sync.dma_start(dst1, x_d[:].bitcast(f32r))

        def win3d(buf, j, d=0):
            base = INT0 + j * 8 * W_PAD + d
            return buf[:, base:base + 8 * W_PAD] \
                .rearrange("p (r c) -> p r c", c=W_PAD)[:, :, 1:65]

        def win3d64(buf, j, parts=128):
            base = INT0 + j * 8 * W_PAD
            return buf[0:parts, base:base + 8 * W_PAD] \
                .rearrange("p (r c) -> p r c", c=W_PAD)[:, :, 1:65]

        # ---- conv1 (row-aligned): lc = W@x + b (f32r), rl (bf16 padded) ---
        rl_t = [t1.tile([128, BUFW], bf16, name=f"rlA{h}") for h in range(2)]
        for h in range(2):
            for j in range(8):
                ps = pp.tile([128, 512], f32, tag="ps", name="ps")
                for dc in range(3):
                    nc.tensor.matmul(ps[:], cwp[:, h, dc, :],
                                     win3d(x2, j, dc - 67),
                                     start=(dc == 0), stop=False)
                for dc in range(3):
                    base = INT0 + j * 8 * W_PAD + 65 + dc
                    rhs = x2[0:64, base:base + 8 * W_PAD] \
                        .rearrange("p (r c) -> p r c", c=W_PAD)[:, :, 1:65]
                    nc.tensor.matmul(ps[:], cws[:, h, dc, :], rhs,
                                     start=False, stop=(dc == 2))
                nc.scalar.activation(lc[h][:, j * 512:(j + 1) * 512], ps[:],
                                     Act.Identity, bias=b2[:, h:h + 1])
                nc.vector.tensor_scalar(
                    win3d(rl_t[h], j), ps[:].rearrange("p (r c) -> p r c", c=64),
                    b2[:, h:h + 1], 0.0, op0=Alu.add, op1=Alu.max)
        for h in range(2):
            nc.gpsimd.memset(rl_t[h][:, 0:INT0], 0.0)
            nc.gpsimd.memset(rl_t[h][:, INT0 + INTN:BUFW], 0.0)
            side = rl_t[h][:, INT0:INT0 + INTN].rearrange("p (r c) -> p r c", c=W_PAD)
            nc.gpsimd.memset(side[:, :, 0:1], 0.0)
            nc.gpsimd.memset(side[:, :, 65:66], 0.0)
        x2_cm.__exit__(None, None, None)

        def conv3x3(rin, rw, tname):
            rt = t2.tile([128, BUFW], bf16, tag="rt", name=tname)
            for ci, (off, n) in enumerate(conv_chunks):
                ps = pp.tile([128, 512], f32, tag="ps", name="ps")
                k = 0
                for kh in range(3):
                    for kw in range(3):
                        d = (kh - 1) * W_PAD + (kw - 1)
                        for h in range(2):
                            nc.tensor.matmul(ps[:, :n], rw[:, kh * 3 + kw, h, :],
                                             rin[h][:, off + d: off + d + n],
                                             start=(k == 0), stop=(k == 17))
                            k += 1
                if ci % 2 == 0:
                    nc.vector.tensor_scalar(rt[:, off:off + n], ps[:, :n],
                                            0.0, None, op0=Alu.max)
                else:
                    nc.scalar.activation(rt[:, off:off + n], ps[:, :n], Act.Relu)
            return rt

        # ---- res1: lc += conv1x1(rt1); rl1 = relu(lc) ---------------------
        rt1 = conv3x3(rl_t, rw1, "rt1")
        rl1 = [t2.tile([128, BUFW], bf16, name=f"rlB{h}") for h in range(2)]
        for h in range(2):
            for j in range(8):
                ps = pp.tile([128, 512], f32, tag="ps", name="ps")
                nc.tensor.matmul(ps[:], rw2[:, 128 * h:128 * h + 128],
                                 win3d64(rt1, j), start=True, stop=True)
                nc.vector.tensor_tensor(lc[h][:, j * 512:(j + 1) * 512], ps[:],
                                        lc[h][:, j * 512:(j + 1) * 512],
                                        op=Alu.add)
                nc.scalar.activation(
                    win3d(rl1[h], j),
                    lc[h][:, j * 512:(j + 1) * 512].rearrange("p (r c) -> p r c", c=64),
                    Act.Relu)
            nc.gpsimd.memset(rl1[h][:, 0:INT0], 0.0)
            nc.gpsimd.memset(rl1[h][:, INT0 + INTN:BUFW], 0.0)
            side = rl1[h][:, INT0:INT0 + INTN].rearrange("p (r c) -> p r c", c=W_PAD)
            nc.gpsimd.memset(side[:, :, 0:1], 0.0)
            nc.gpsimd.memset(side[:, :, 65:66], 0.0)
        t1_cm.__exit__(None, None, None)

        # res2 weights reuse the res1 slots
        rw1b = cw.tile([128, 9, 2, 128], bf16, tag="rw1", name="rw1b")
        nc.sync.dma_start(rw1b[:], rw1b_d[:])
        rw2b = cw.tile([128, 256], bf16, tag="rw2", name="rw2b")
        nc.sync.dma_start(rw2b[:], rw2b_d[:])

        # ---- res2: lc += conv1x1(rt2), channel sums ride the update -------
        rt2 = conv3x3(rl1, rw1b, "rt2")
        sums = [ev.tile([128, 8], f32, tag=f"sum{h}", name=f"sum{h}", bufs=1)
                for h in range(2)]
        for h in range(2):
            for j in range(8):
                ps = pp.tile([128, 512], f32, tag="ps", name="ps")
                nc.tensor.matmul(ps[:], rw2b[:, 128 * h:128 * h + 128],
                                 win3d64(rt2, j), start=True, stop=True)
                nc.vector.scalar_tensor_tensor(
                    lc[h][:, j * 512:(j + 1) * 512], ps[:], 0.0,
                    lc[h][:, j * 512:(j + 1) * 512], op0=Alu.add, op1=Alu.add,
                    accum_out=sums[h][:, j:j + 1])
        t2_cm.__exit__(None, None, None)
        cw_cm.__exit__(None, None, None)
        zp_cm = tc.tile_pool(name="zp", bufs=2)
        zp = zp_cm.__enter__()

        # ---- CBAM channel attention --------------------------------------
        avmx = []
        for h in range(2):
            ssum = ev.tile([128, 1], f32, tag="ssum", name="ssum", bufs=2)
            nc.vector.tensor_reduce(ssum[:], sums[h][:], axis=mybir.AxisListType.X,
                                    op=Alu.add)
            am = ev.tile([128, 2], f32r, tag=f"avmx{h}", name=f"avmx{h}", bufs=1)
            nc.vector.tensor_scalar(am[:, 0:1], ssum[:], 1.0 / NPIX, None,
                                    op0=Alu.mult)
            nc.vector.tensor_reduce(am[:, 1:2], lc[h][:], axis=mybir.AxisListType.X,
                                    op=Alu.max)
            avmx.append(am)
        psfc = pp.tile([16, 2], f32, tag="ps", name="psfc")
        for h in range(2):
            nc.tensor.matmul(psfc[:], caw1[:, h, :], avmx[h][:],
                             start=(h == 0), stop=(h == 1))
        fc1 = ev.tile([16, 2], f32r, tag="fc1", name="fc1", bufs=1)
        nc.vector.tensor_scalar(fc1[:], psfc[:], 0.0, None, op0=Alu.max)
        schl = []
        schq = []
        for h in range(2):
            ps2 = pp.tile([128, 2], f32, tag="ps", name="ps2")
            nc.tensor.matmul(ps2[:], caw2[:, 128 * h:128 * h + 128], fc1[:],
                             start=True, stop=True)
            fcs = ev.tile([128, 2], f32, tag=f"fcs{h}", name=f"fcs{h}", bufs=1)
            nc.scalar.copy(fcs[:], ps2[:])
            sc = ev.tile([128, 1], f32, tag=f"sch{h}", name=f"sch{h}", bufs=1)
            nc.vector.tensor_tensor(sc[:], fcs[:, 0:1], fcs[:, 1:2], op=Alu.add)
            nc.scalar.activation(sc[:], sc[:], Act.Sigmoid)
            scl = ev.tile([128, 1], f32, tag=f"schl{h}", name=f"schl{h}", bufs=1)
            nc.vector.tensor_scalar(scl[:], sc[:], 1.0 / L, None, op0=Alu.mult)
            scq = ev.tile([128, 1], f32r, tag=f"schq{h}", name=f"schq{h}", bufs=1)
            nc.vector.tensor_scalar(scq[:], sc[:], 1.0 / 256.0, None, op0=Alu.mult)
            schl.append(scl)
            schq.append(scq)
            schq.append(sc)

        # ---- SA: channel mean (PE) + max (gpsimd all-reduce) --------------
        for j in range(8):
            js = slice(j * 512, (j + 1) * 512)
            psm = pp.tile([1, 512], f32, tag="ps", name="psm")
            for h in range(2):
                nc.tensor.matmul(psm[:], schq[2 * h][:], lc[h][:, js],
                                 start=(h == 0), stop=(h == 1))
            sm = ev.tile([1, 512], f32, tag="sm", name="sm", bufs=2)
            nc.scalar.copy(sm[:], psm[:])
            nc.sync.dma_start(sab_d[0:1, js], sm[:])
            m0 = ev.tile([128, 512], bf16, tag="m0", name="m0", bufs=2)
            nc.vector.tensor_scalar(m0[:], lc[0][:, js], schq[1][:], None,
                                    op0=Alu.mult)
            m1 = ev.tile([128, 512], bf16, tag="m1", name="m1", bufs=2)
            nc.vector.tensor_scalar(m1[:], lc[1][:, js], schq[3][:], None,
                                    op0=Alu.mult)
            mh = ev.tile([128, 512], bf16, tag="mh", name="mh", bufs=2)
            nc.vector.tensor_tensor(mh[:], m0[:], m1[:], op=Alu.max)
            pr = ev.tile([128, 512], bf16, tag="pr", name="pr", bufs=2)
            nc.gpsimd.partition_all_reduce(pr[:], mh[:], channels=128,
                                           reduce_op=bass_isa.ReduceOp.max)
            sx = ev.tile([1, 512], f32, tag="sx", name="sx", bufs=2)
            nc.vector.tensor_copy(sx[:], pr[0:1, :])
            nc.sync.dma_start(sab_d[1:2, js], sx[:])
        msh = []
        for kh in range(3):
            mt = ev.tile([64, 2, 66], f32, tag=f"msh{kh}", name=f"msh{kh}", bufs=1)
            nc.vector.memset(mt[:], 0.0)
            sr = sab_d[:].rearrange("c (h w) -> h c w", w=64)
            if kh == 0:
                nc.sync.dma_start(mt[1:64, :, 1:65], sr[0:63])
            elif kh == 1:
                nc.sync.dma_start(mt[0:64, :, 1:65], sr[0:64])
            else:
                nc.sync.dma_start(mt[0:63, :, 1:65], sr[1:64])
            msh.append(mt)
        acc = [ev.tile([64, 64], f32, tag=f"sacc{i}", name=f"sacc{i}", bufs=1)
               for i in range(2)]
        k = 0
        for ch in range(2):
            for kh in range(3):
                for kw in range(3):
                    w = float(sa_w[0, ch, kh, kw])
                    src = msh[kh][0:64, ch, kw:kw + 64]
                    if k == 0:
                        nc.vector.tensor_scalar(acc[0][:], src, w, None,
                                                op0=Alu.mult)
                    else:
                        nc.vector.scalar_tensor_tensor(
                            acc[k % 2][:], src, w, acc[(k + 1) % 2][:],
                            op0=Alu.mult, op1=Alu.add)
                    k += 1
        ssp = ev.tile([64, 64], f32r, tag="ssp", name="ssp", bufs=1)
        nc.scalar.activation(ssp[:], acc[(k + 1) % 2][:], Act.Sigmoid)
        nc.sync.dma_start(ssb_d[:].rearrange("o (h w) -> (o h) w", w=64),
                          ssp[:].bitcast(f32))
        ssprow = ev.tile([1, NPIX], f32r, tag="ssprow", name="ssprow", bufs=1)
        nc.sync.dma_start(ssprow[:], ssb_d[:].bitcast(f32r))

        # ---- lam = lc * (s_ch/L) * s_sp (fp32) ----------------------------
        lam = [[None] * 8 for _ in range(2)]
        for j in range(8):
            psb = pp.tile([128, 512], f32, tag="ps", name="psb")
            nc.tensor.matmul(psb[:], ones1[:], ssprow[:, j * 512:(j + 1) * 512],
                             start=True, stop=True)
            for h in range(2):
                lslice = lc[h][:, j * 512:(j + 1) * 512]
                nc.vector.scalar_tensor_tensor(
                    lslice, lslice, schl[h][:],
                    psb[:], op0=Alu.mult, op1=Alu.mult)
                lam[h][j] = lslice

        # ---- LISTA: groups of 4 chunks, t-major round-robin ---------------
        for g in range(2):
            jset = range(4 * g, 4 * g + 4)
            zprev = {j: [None, None] for j in jset}
            for t in range(num_iters + 1):
                for j in jset:
                    js = slice(j * 512, (j + 1) * 512)
                    znew = [None, None]
                    for h in range(2):
                        ps = pp.tile([128, 512], f32, tag="ps", name="psv")
                        if t == 0:
                            nc.tensor.matmul(ps[:], dctF[:, 128 * h:128 * h + 128],
                                             xF[:, js], start=True, stop=True)
                            nc.scalar.activation(yLr[h][:, js], ps[:], Act.Copy,
                                                 scale=1.0 / L)
                        else:
                            nc.scalar.copy(ps[:], yLr[h][:, js])
                            nc.tensor.matmul(ps[:], s_t[:, 0, 128 * h:128 * h + 128],
                                             zprev[j][0][:], start=False, stop=False,
                                             skip_group_check=True)
                            nc.tensor.matmul(ps[:], s_t[:, 1, 128 * h:128 * h + 128],
                                             zprev[j][1][:], start=False, stop=True,
                                             skip_group_check=True)
                        z = zp.tile([128, 512], f32r, tag=f"z{j % 4}_{h}",
                                    name=f"z{j % 4}_{h}")
                        nc.vector._custom_dve(SOFT, out=z[:], in0=ps[:],
                                              in1=lam[h][j][:])
                        znew[h] = z
                    zprev[j] = znew
            for j in jset:
                js = slice(j * 512, (j + 1) * 512)
                for h in range(2):
                    nc.sync.dma_start(zo_d[128 * h:128 * h + 128, js],
                                      zprev[j][h][:].bitcast(f32))
                psr = pp.tile([64, 512], f32, tag="ps", name="psr")
                for h in range(2):
                    nc.tensor.matmul(psr[:], dctt[:, h, :], zprev[j][h][:],
                                     start=(h == 0), stop=(h == 1))
                rc = ev.tile([64, 512], f32, tag="rc", name="rc", bufs=2)
                nc.scalar.copy(rc[:], psr[:])
                nc.sync.dma_start(ro_d[:, js], rc[:])

        zp_cm.__exit__(None, None, None)
        late_cm.__exit__(None, None, None)
        pp_cm.__exit__(None, None, None)
        ev_cm.__exit__(None, None, None)
        wp_cm.__exit__(None, None, None)

    nc.compile()
    return nc


def _host_prep(inputs):
    b16 = ml_dtypes.bfloat16
    cw = np.ascontiguousarray(inputs["conv_w"], dtype=np.float32)
    cwp = np.zeros((128, 2, 3, 128), np.float32)
    cws = np.zeros((64, 2, 3, 128), np.float32)
    for h in range(2):
        for dc in range(3):
            cwp[0:64, h, dc, :] = cw[128 * h:128 * h + 128, :, 0, dc].T
            cwp[64:128, h, dc, :] = cw[128 * h:128 * h + 128, :, 1, dc].T
            cws[:, h, dc, :] = cw[128 * h:128 * h + 128, :, 2, dc].T
    b2 = np.ascontiguousarray(
        np.asarray(inputs["conv_b"], np.float32).reshape(2, 128).T)

    def res_pack(w1, w2):
        w1 = np.asarray(w1, np.float32)
        r1 = np.zeros((128, 9, 2, 128), np.float32)
        for kh in range(3):
            for kw in range(3):
                for h in range(2):
                    r1[:, kh * 3 + kw, h, :] = w1[:, 128 * h:128 * h + 128, kh, kw].T
        r2 = np.ascontiguousarray(np.asarray(w2, np.float32)[:, :, 0, 0].T)
        return r1.astype(b16), r2.astype(b16)

    rw1a, rw2a = res_pack(inputs["res1_w1"], inputs["res1_w2"])
    rw1b, rw2b = res_pack(inputs["res2_w1"], inputs["res2_w2"])
    caw1 = np.ascontiguousarray(
        np.asarray(inputs["ca_w1"], np.float32).T.reshape(2, 128, 16)
        .transpose(1, 0, 2))
    caw2 = np.ascontiguousarray(np.asarray(inputs["ca_w2"], np.float32).T)
    D = np.asarray(inputs["Dict"], np.float32)
    L = float(np.asarray(inputs["L"]))
    S = np.ascontiguousarray(
        (np.eye(256, dtype=np.float32) - (D.T @ D) / np.float32(L)).T
        .reshape(2, 128, 256).transpose(1, 0, 2))
    dctt = np.ascontiguousarray(D.T.reshape(2, 128, 64).transpose(1, 0, 2))
    return dict(cwp=cwp, cws=cws, b2=b2,
                rw1a=rw1a, rw2a=rw2a, rw1b=rw1b, rw2b=rw2b, caw1=caw1,
                caw2=caw2, S=S, Dct=D, DctT=dctt,
                ident=np.eye(128, dtype=np.float32)), L


def kernel(**inputs):
    num_iters = int(np.asarray(inputs["num_iters"]))
    sa_w = np.asarray(inputs["sa_w"], np.float32)
    weights, L = _host_prep(inputs)
    nc = build_nc(num_iters, L, sa_w)
    x = np.ascontiguousarray(np.asarray(inputs["x"], np.float32))
    in_maps = [dict(weights, x=x[b]) for b in range(8)]
    res = run_bass_kernel_spmd(nc, in_maps, core_ids=list(range(8)))
    z = np.stack([res.results[b]["zo"].reshape(256, 64, 64) for b in range(8)])
    rec = np.stack([res.results[b]["ro"].reshape(64, 64, 64) for b in range(8)])
    return (z, rec, np.asarray(inputs["Dict"], np.float32))


if __name__ == "__main__":
    d = np.load("/root/problem/ref_cache.npz")
    ins = {k: d[k] for k in ["x", "conv_w", "conv_b", "res1_w1", "res1_w2",
                             "res2_w1", "res2_w2", "ca_w1", "ca_w2", "sa_w",
                             "Dict", "L", "num_iters"]}
    out = kernel(**ins)
    for i, name in enumerate(["z", "recon", "Dict"]):
        ref = d[f"out{i}"]
        got = out[i]
        num = np.abs(got - ref).max()
        den = np.abs(ref).max()
        print(f"{name}: absmax diff {num:.3e}  scale {den:.3e}  rel {num/den:.3e}")


# revision 21
# speedup vs baseline: 1.2462x; 1.0007x over previous
"""AttentiveLISTA Trainium2 kernel — pure data parallel over 8 NeuronCores.

Per core (B=1 image, C=64, H=W=64, A=256):
  conv3x3(64->256)+bias -> 2x residual blocks -> CBAM (channel+spatial
  attention) -> per-pixel LISTA (z0 = soft(y); 16x z = soft(z@S + y/L)).

Implementation notes:
  * channels on SBUF partitions, flattened padded 66x66 image on the free dim
    (base offset +2, buffers [*, 4360]); 3x3 convs = shifted matmuls (conv1
    pairs taps kh=0/kh=1 per K=128 chunk via a second +66-shifted x copy).
  * The conv/CBAM tower only produces the soft-thresholds lam (the LISTA data
    path is x/Dict/S alone), so the tower runs in bf16: same 1 cyc/row PE
    speed, half the SBUF, and a ~1e-2-relative lam error that perturbs z by
    ~|dlam| ~ 3e-5 absolute.
  * LISTA matmuls in float32r (~13-bit-mantissa fp32, 1 PE cycle/row).
  * LISTA soft-threshold = one fused custom DVE op per tile:
        out = v > 0 ? relu(v - lam) : -relu(-v - lam)
    reading v straight from PSUM, writing z as f32r (matmul-ready).
  * y/L rides the loop's PSUM accumulation as a 3rd K=64 matmul (Dict/L @ x),
    so no separate add pass exists.
"""
import os
import sys

for _p in ("/opt/trn_rl_repo", "/opt/trn_rl_repo/concourse"):
    if _p not in sys.path:
        sys.path.insert(0, _p)

import numpy as np
import ml_dtypes

import concourse.bass as bass
from concourse import bacc, mybir, bass_isa
import concourse.tile as tile
from concourse.bass_utils import run_bass_kernel_spmd

f32 = mybir.dt.float32
f32r = mybir.dt.float32r
bf16 = mybir.dt.bfloat16
Alu = mybir.AluOpType
Act = mybir.ActivationFunctionType

W_PAD = 66           # padded row width
BASE = 2             # flat offset of padded pixel 0 inside the [*, 4360] bufs
BUFW = 4360
INT0 = BASE + W_PAD  # 68: first interior-row flat position
INTN = 64 * W_PAD    # 4224: contiguous interior span (rows 1..64 incl. side pads)
NPIX = 4096

_SOFT_OP = None


def _register_softshrink():
    global _SOFT_OP
    if _SOFT_OP is not None:
        return _SOFT_OP
    import concourse.dve_ops as dve_ops
    from concourse.dve_spec import Spec, Src0, Src1, Zero, relu, select, lower
    from concourse.dve_uop import DveOpSpec

    name = "SOFTSHRINK_ANT"
    for op in dve_ops.OPS:
        if op.name == name:
            _SOFT_OP = op
            return op

    def _reference(in0, in1, s0, s1, imm2):
        v = in0.astype(np.float32) * np.float32(s0)
        return np.where(v > 0, np.maximum(v - in1, 0.0),
                        -np.maximum(-v - in1, 0.0)).astype(np.float32)

    from concourse.dve_spec import C0, maxx
    v = Src0 * C0
    r = relu(maxx(v, Zero - v) - Src1)
    body = select(v > Zero, r, Zero - r)
    spec = Spec(body=body, reference=_reference)
    row = dve_ops._CUSTOM_DVE_ROW_BASE + len(dve_ops.OPS)
    assert row < 0x20
    shas = {}
    for ver in ("v3", "v4"):
        tmp = DveOpSpec(name=name, opcode=row, uops=lower(spec, ver=ver),
                        rd1_en=True)
        shas[ver] = tmp.sha(ver)
    op = dve_ops.DveOp(name, spec, subdim=False, uops_sha=shas)
    dve_ops.OPS.append(op)
    dve_ops.CUSTOM_DVE_SPECS[name] = spec
    dve_ops._SUB_OPCODE_FOR_NAME[name] = row
    _SOFT_OP = op
    return op


def _chunks512(start, span):
    out = []
    o = start
    while o < start + span:
        n = min(512, start + span - o)
        out.append((o, n))
        o += n
    return out


def build_nc(num_iters, L, sa_w):
    SOFT = _register_softshrink()
    nc = bacc.Bacc("TRN2", target_bir_lowering=False, debug=False)

    x_d = nc.dram_tensor("x", [64, 64, 64], f32, kind="ExternalInput")
    cwp_d = nc.dram_tensor("cwp", [128, 2, 3, 128], f32, kind="ExternalInput")
    cws_d = nc.dram_tensor("cws", [64, 2, 3, 128], f32, kind="ExternalInput")
    b2_d = nc.dram_tensor("b2", [128, 2], f32, kind="ExternalInput")
    rw1a_d = nc.dram_tensor("rw1a", [128, 9, 2, 128], bf16, kind="ExternalInput")
    rw2a_d = nc.dram_tensor("rw2a", [128, 256], bf16, kind="ExternalInput")
    rw1b_d = nc.dram_tensor("rw1b", [128, 9, 2, 128], bf16, kind="ExternalInput")
    rw2b_d = nc.dram_tensor("rw2b", [128, 256], bf16, kind="ExternalInput")
    caw1_d = nc.dram_tensor("caw1", [128, 2, 16], f32, kind="ExternalInput")
    caw2_d = nc.dram_tensor("caw2", [16, 256], f32, kind="ExternalInput")
    s_d = nc.dram_tensor("S", [128, 2, 256], f32, kind="ExternalInput")
    dict_d = nc.dram_tensor("Dct", [64, 256], f32, kind="ExternalInput")
    ident_d = nc.dram_tensor("ident", [128, 128], f32, kind="ExternalInput")
    dictt_d = nc.dram_tensor("DctT", [128, 2, 64], f32, kind="ExternalInput")

    zo_d = nc.dram_tensor("zo", [256, NPIX], f32, kind="ExternalOutput")
    ro_d = nc.dram_tensor("ro", [64, NPIX], f32, kind="ExternalOutput")
    sab_d = nc.dram_tensor("sab", [2, 4096], f32)
    ssb_d = nc.dram_tensor("ssb", [1, 4096], f32)

    conv_chunks = _chunks512(INT0, INTN)

    with tile.TileContext(nc) as tc:
        wp_cm = tc.tile_pool(name="wp", bufs=1)
        ev_cm = tc.tile_pool(name="ev", bufs=4)
        pp_cm = tc.tile_pool(name="pp", bufs=8, space="PSUM")
        late_cm = tc.tile_pool(name="late", bufs=1)
        cw_cm = tc.tile_pool(name="cw", bufs=1)
        t2_cm = tc.tile_pool(name="t2", bufs=1)
        t1_cm = tc.tile_pool(name="t1", bufs=1)
        x2_cm = tc.tile_pool(name="x2p", bufs=1)
        wp = wp_cm.__enter__()
        ev = ev_cm.__enter__()
        pp = pp_cm.__enter__()
        late = late_cm.__enter__()
        cw = cw_cm.__enter__()
        t2 = t2_cm.__enter__()
        t1 = t1_cm.__enter__()

        # ---- long-lived small weights ------------------------------------
        b2 = wp.tile([128, 2], f32, name="b2")
        nc.sync.dma_start(b2[:], b2_d[:])
        caw1 = wp.tile([128, 2, 16], f32r, name="caw1")
        nc.sync.dma_start(caw1[:], caw1_d[:].bitcast(f32r))
        caw2 = wp.tile([16, 256], f32r, name="caw2")
        nc.sync.dma_start(caw2[:], caw2_d[:].bitcast(f32r))
        s_t = wp.tile([128, 2, 256], f32r, name="s_t")
        nc.sync.dma_start(s_t[:], s_d[:].bitcast(f32r))
        dctF = wp.tile([64, 256], f32, name="dctF")
        nc.sync.dma_start(dctF[:], dict_d[:])
        ident = wp.tile([128, 128], f32r, name="ident")
        nc.sync.dma_start(ident[:], ident_d[:].bitcast(f32r))
        dctt = wp.tile([128, 2, 64], f32r, name="dctt")
        nc.sync.dma_start(dctt[:], dictt_d[:].bitcast(f32r))
        ones1 = wp.tile([1, 128], f32r, name="ones1")
        nc.vector.memset(ones1.bitcast(f32)[:], 1.0)
        xF = wp.tile([64, NPIX], f32, name="xF")
        nc.sync.dma_start(xF[:], x_d[:].rearrange("c h w -> c (h w)"))

        # lam-path carrier: l -> l1 -> l2, updated in place (compact f32r)
        lc = [late.tile([128, NPIX], f32r, name=f"lc{h}") for h in range(2)]
        yLr = [late.tile([128, NPIX], f32r, name=f"yLr{h}") for h in range(2)]

        # ---- conv1 weights (f32r) + res weights (bf16) --------------------
        cwp = cw.tile([128, 2, 3, 128], f32r, name="cwp")
        nc.sync.dma_start(cwp[:], cwp_d[:].bitcast(f32r))
        cws = cw.tile([64, 2, 3, 128], f32r, name="cws")
        nc.sync.dma_start(cws[:], cws_d[:].bitcast(f32r))
        rw1 = cw.tile([128, 9, 2, 128], bf16, tag="rw1", name="rw1")
        nc.sync.dma_start(rw1[:], rw1a_d[:])
        rw2 = cw.tile([128, 256], bf16, tag="rw2", name="rw2")
        nc.sync.dma_start(rw2[:], rw2a_d[:])

        # ---- y-phase: exact fp32 y = x @ Dict, evicted as yLr = y/L -------
        for h in range(2):
            for j in range(8):
                js = slice(j * 512, (j + 1) * 512)
                ps = pp.tile([128, 512], f32, tag="ps", name="psy")
                nc.tensor.matmul(ps[:], dctF[:, 128 * h:128 * h + 128],
                                 xF[:, js], start=True, stop=True)
                nc.scalar.activation(yLr[h][:, js], ps[:], Act.Copy,
                                     scale=1.0 / L)

        x2p = x2_cm.__enter__()
        x2 = x2p.tile([128, BUFW], f32r, name="x2")
        nc.vector.memset(x2.bitcast(f32)[0:64, :], 0.0)
        nc.gpsimd.memset(x2.bitcast(f32)[64:128, :], 0.0)
        dst0 = x2[0:64, INT0:INT0 + INTN].rearrange("p (r c) -> p r c", c=W_PAD)[:, :, 1:65]
        nc.sync.dma_start(dst0, x_d[:].bitcast(f32r))
        dst1 = x2[64:128, BASE:BASE + INTN].rearrange("p (r c) -> p r c", c=W_PAD)[:, :, 1:65]
        nc.sync.dma_start(dst1, x_d[:].bitcast(f32r))

        def win3d(buf, j, d=0):
            base = INT0 + j * 8 * W_PAD + d
            return buf[:, base:base + 8 * W_PAD] \
                .rearrange("p (r c) -> p r c", c=W_PAD)[:, :, 1:65]

        def win3d64(buf, j, parts=128):
            base = INT0 + j * 8 * W_PAD
            return buf[0:parts, base:base + 8 * W_PAD] \
                .rearrange("p (r c) -> p r c", c=W_PAD)[:, :, 1:65]

        # ---- conv1 (row-aligned): lc = W@x + b (f32r), rl (bf16 padded) ---
        rl_t = [t1.tile([128, BUFW], bf16, name=f"rlA{h}") for h in range(2)]
        for h in range(2):
            for j in range(8):
                ps = pp.tile([128, 512], f32, tag="ps", name="ps")
                for dc in range(3):
                    nc.tensor.matmul(ps[:], cwp[:, h, dc, :],
                                     win3d(x2, j, dc - 67),
                                     start=(dc == 0), stop=False)
                for dc in range(3):
                    base = INT0 + j * 8 * W_PAD + 65 + dc
                    rhs = x2[0:64, base:base + 8 * W_PAD] \
                        .rearrange("p (r c) -> p r c", c=W_PAD)[:, :, 1:65]
                    nc.tensor.matmul(ps[:], cws[:, h, dc, :], rhs,
                                     start=False, stop=(dc == 2))
                nc.scalar.activation(lc[h][:, j * 512:(j + 1) * 512], ps[:],
                                     Act.Identity, bias=b2[:, h:h + 1])
                nc.vector.tensor_scalar(
                    win3d(rl_t[h], j), ps[:].rearrange("p (r c) -> p r c", c=64),
                    b2[:, h:h + 1], 0.0, op0=Alu.add, op1=Alu.max)
        for h in range(2):
            nc.gpsimd.memset(rl_t[h][:, 0:INT0], 0.0)
            nc.gpsimd.memset(rl_t[h][:, INT0 + INTN:BUFW], 0.0)
            side = rl_t[h][:, INT0:INT0 + INTN].rearrange("p (r c) -> p r c", c=W_PAD)
            nc.gpsimd.memset(side[:, :, 0:1], 0.0)
            nc.gpsimd.memset(side[:, :, 65:66], 0.0)
        x2_cm.__exit__(None, None, None)

        def conv3x3(rin, rw, tname):
            rt = t2.tile([128, BUFW], bf16, tag="rt", name=tname)
            for ci, (off, n) in enumerate(conv_chunks):
                ps = pp.tile([128, 512], f32, tag="ps", name="ps")
                k = 0
                for kh in range(3):
                    for kw in range(3):
                        d = (kh - 1) * W_PAD + (kw - 1)
                        for h in range(2):
                            nc.tensor.matmul(ps[:, :n], rw[:, kh * 3 + kw, h, :],
                                             rin[h][:, off + d: off + d + n],
                                             start=(k == 0), stop=(k == 17))
                            k += 1
                if ci % 2 == 0:
                    nc.vector.tensor_scalar(rt[:, off:off + n], ps[:, :n],
                                            0.0, None, op0=Alu.max)
                else:
                    nc.scalar.activation(rt[:, off:off + n], ps[:, :n], Act.Relu)
            return rt

        # ---- res1: lc += conv1x1(rt1); rl1 = relu(lc) ---------------------
        rt1 = conv3x3(rl_t, rw1, "rt1")
        rl1 = [t2.tile([128, BUFW], bf16, name=f"rlB{h}") for h in range(2)]
        for h in range(2):
            for j in range(8):
                ps = pp.tile([128, 512], f32, tag="ps", name="ps")
                nc.tensor.matmul(ps[:], rw2[:, 128 * h:128 * h + 128],
                                 win3d64(rt1, j), start=True, stop=True)
                nc.vector.tensor_tensor(lc[h][:, j * 512:(j + 1) * 512], ps[:],
                                        lc[h][:, j * 512:(j + 1) * 512],
                                        op=Alu.add)
                nc.scalar.activation(
                    win3d(rl1[h], j),
                    lc[h][:, j * 512:(j + 1) * 512].rearrange("p (r c) -> p r c", c=64),
                    Act.Relu)
            nc.gpsimd.memset(rl1[h][:, 0:INT0], 0.0)
            nc.gpsimd.memset(rl1[h][:, INT0 + INTN:BUFW], 0.0)
            side = rl1[h][:, INT0:INT0 + INTN].rearrange("p (r c) -> p r c", c=W_PAD)
            nc.gpsimd.memset(side[:, :, 0:1], 0.0)
            nc.gpsimd.memset(side[:, :, 65:66], 0.0)
        t1_cm.__exit__(None, None, None)

        # res2 weights reuse the res1 slots
        rw1b = cw.tile([128, 9, 2, 128], bf16, tag="rw1", name="rw1b")
        nc.sync.dma_start(rw1b[:], rw1b_d[:])
        rw2b = cw.tile([128, 256], bf16, tag="rw2", name="rw2b")
        nc.sync.dma_start(rw2b[:], rw2b_d[:])

        # ---- res2: lc += conv1x1(rt2), channel sums ride the update -------
        rt2 = conv3x3(rl1, rw1b, "rt2")
        sums = [ev.tile([128, 8], f32, tag=f"sum{h}", name=f"sum{h}", bufs=1)
                for h in range(2)]
        for h in range(2):
            for j in range(8):
                ps = pp.tile([128, 512], f32, tag="ps", name="ps")
                nc.tensor.matmul(ps[:], rw2b[:, 128 * h:128 * h + 128],
                                 win3d64(rt2, j), start=True, stop=True)
                nc.vector.scalar_tensor_tensor(
                    lc[h][:, j * 512:(j + 1) * 512], ps[:], 0.0,
                    lc[h][:, j * 512:(j + 1) * 512], op0=Alu.add, op1=Alu.add,
                    accum_out=sums[h][:, j:j + 1])
        t2_cm.__exit__(None, None, None)
        cw_cm.__exit__(None, None, None)
        zp_cm = tc.tile_pool(name="zp", bufs=2)
        zp = zp_cm.__enter__()

        # ---- CBAM channel attention --------------------------------------
        avmx = []
        for h in range(2):
            ssum = ev.tile([128, 1], f32, tag="ssum", name="ssum", bufs=2)
            nc.vector.tensor_reduce(ssum[:], sums[h][:], axis=mybir.AxisListType.X,
                                    op=Alu.add)
            am = ev.tile([128, 2], f32r, tag=f"avmx{h}", name=f"avmx{h}", bufs=1)
            nc.vector.tensor_scalar(am[:, 0:1], ssum[:], 1.0 / NPIX, None,
                                    op0=Alu.mult)
            nc.vector.tensor_reduce(am[:, 1:2], lc[h][:], axis=mybir.AxisListType.X,
                                    op=Alu.max)
            avmx.append(am)
        psfc = pp.tile([16, 2], f32, tag="ps", name="psfc")
        for h in range(2):
            nc.tensor.matmul(psfc[:], caw1[:, h, :], avmx[h][:],
                             start=(h == 0), stop=(h == 1))
        fc1 = ev.tile([16, 2], f32r, tag="fc1", name="fc1", bufs=1)
        nc.vector.tensor_scalar(fc1[:], psfc[:], 0.0, None, op0=Alu.max)
        schl = []
        schq = []
        for h in range(2):
            ps2 = pp.tile([128, 2], f32, tag="ps", name="ps2")
            nc.tensor.matmul(ps2[:], caw2[:, 128 * h:128 * h + 128], fc1[:],
                             start=True, stop=True)
            fcs = ev.tile([128, 2], f32, tag=f"fcs{h}", name=f"fcs{h}", bufs=1)
            nc.scalar.copy(fcs[:], ps2[:])
            sc = ev.tile([128, 1], f32, tag=f"sch{h}", name=f"sch{h}", bufs=1)
            nc.vector.tensor_tensor(sc[:], fcs[:, 0:1], fcs[:, 1:2], op=Alu.add)
            nc.scalar.activation(sc[:], sc[:], Act.Sigmoid)
            scl = ev.tile([128, 1], f32, tag=f"schl{h}", name=f"schl{h}", bufs=1)
            nc.vector.tensor_scalar(scl[:], sc[:], 1.0 / L, None, op0=Alu.mult)
            scq = ev.tile([128, 1], f32r, tag=f"schq{h}", name=f"schq{h}", bufs=1)
            nc.vector.tensor_scalar(scq[:], sc[:], 1.0 / 256.0, None, op0=Alu.mult)
            schl.append(scl)
            schq.append(scq)
            schq.append(sc)

        # ---- SA: channel mean (PE) + max (gpsimd all-reduce) --------------
        for j in range(8):
            js = slice(j * 512, (j + 1) * 512)
            psm = pp.tile([1, 512], f32, tag="ps", name="psm")
            for h in range(2):
                nc.tensor.matmul(psm[:], schq[2 * h][:], lc[h][:, js],
                                 start=(h == 0), stop=(h == 1))
            sm = ev.tile([1, 512], f32, tag="sm", name="sm", bufs=2)
            nc.scalar.copy(sm[:], psm[:])
            nc.sync.dma_start(sab_d[0:1, js], sm[:])
            m0 = ev.tile([128, 512], bf16, tag="m0", name="m0", bufs=2)
            nc.vector.tensor_scalar(m0[:], lc[0][:, js], schq[1][:], None,
                                    op0=Alu.mult)
            m1 = ev.tile([128, 512], bf16, tag="m1", name="m1", bufs=2)
            nc.vector.tensor_scalar(m1[:], lc[1][:, js], schq[3][:], None,
                                    op0=Alu.mult)
            mh = ev.tile([128, 512], bf16, tag="mh", name="mh", bufs=2)
            nc.vector.tensor_tensor(mh[:], m0[:], m1[:], op=Alu.max)
            pr = ev.tile([128, 512], bf16, tag="pr", name="pr", bufs=2)
            nc.gpsimd.partition_all_reduce(pr[:], mh[:], channels=128,
                                           reduce_op=bass_isa.ReduceOp.max)
            sx = ev.tile([1, 512], f32, tag="sx", name="sx", bufs=2)
            nc.vector.tensor_copy(sx[:], pr[0:1, :])
            nc.sync.dma_start(sab_d[1:2, js], sx[:])
        msh = []
        for kh in range(3):
            mt = ev.tile([64, 2, 66], f32, tag=f"msh{kh}", name=f"msh{kh}", bufs=1)
            nc.vector.memset(mt[:], 0.0)
            sr = sab_d[:].rearrange("c (h w) -> h c w", w=64)
            if kh == 0:
                nc.sync.dma_start(mt[1:64, :, 1:65], sr[0:63])
            elif kh == 1:
                nc.sync.dma_start(mt[0:64, :, 1:65], sr[0:64])
            else:
                nc.sync.dma_start(mt[0:63, :, 1:65], sr[1:64])
            msh.append(mt)
        acc = [ev.tile([64, 64], f32, tag=f"sacc{i}", name=f"sacc{i}", bufs=1)
               for i in range(2)]
        k = 0
        for ch in range(2):
            for kh in range(3):
                for kw in range(3):
                    w = float(sa_w[0, ch, kh, kw])
                    src = msh[kh][0:64, ch, kw:kw + 64]
                    if k == 0:
                        nc.vector.tensor_scalar(acc[0][:], src, w, None,
                                                op0=Alu.mult)
                    else:
                        nc.vector.scalar_tensor_tensor(
                            acc[k % 2][:], src, w, acc[(k + 1) % 2][:],
                            op0=Alu.mult, op1=Alu.add)
                    k += 1
        ssp = ev.tile([64, 64], f32r, tag="ssp", name="ssp", bufs=1)
        nc.scalar.activation(ssp[:], acc[(k + 1) % 2][:], Act.Sigmoid)
        nc.sync.dma_start(ssb_d[:].rearrange("o (h w) -> (o h) w", w=64),
                          ssp[:].bitcast(f32))
        ssprow = ev.tile([1, NPIX], f32r, tag="ssprow", name="ssprow", bufs=1)
        nc.sync.dma_start(ssprow[:], ssb_d[:].bitcast(f32r))

        # ---- lam = lc * (s_ch/L) * s_sp (fp32) ----------------------------
        lam = [[None] * 8 for _ in range(2)]
        for j in range(8):
            psb = pp.tile([128, 512], f32, tag="ps", name="psb")
            nc.tensor.matmul(psb[:], ones1[:], ssprow[:, j * 512:(j + 1) * 512],
                             start=True, stop=True)
            for h in range(2):
                lslice = lc[h][:, j * 512:(j + 1) * 512]
                nc.vector.scalar_tensor_tensor(
                    lslice, lslice, schl[h][:],
                    psb[:], op0=Alu.mult, op1=Alu.mult)
                lam[h][j] = lslice

        # ---- LISTA: groups of 4 chunks, t-major round-robin ---------------
        for g in range(2):
            jset = range(4 * g, 4 * g + 4)
            zprev = {j: [None, None] for j in jset}
            for t in range(num_iters + 1):
                for j in jset:
                    js = slice(j * 512, (j + 1) * 512)
                    znew = [None, None]
                    for h in range(2):
                        ps = pp.tile([128, 512], f32, tag="ps", name="psv")
                        if t == 0:
                            nc.tensor.matmul(ps[:], ident[:], yLr[h][:, js],
                                             start=True, stop=True)
                        else:
                            nc.scalar.copy(ps[:], yLr[h][:, js])
                            nc.tensor.matmul(ps[:], s_t[:, 0, 128 * h:128 * h + 128],
                                             zprev[j][0][:], start=False, stop=False,
                                             skip_group_check=True)
                            nc.tensor.matmul(ps[:], s_t[:, 1, 128 * h:128 * h + 128],
                                             zprev[j][1][:], start=False, stop=True,
                                             skip_group_check=True)
                        z = zp.tile([128, 512], f32r, tag=f"z{j % 4}_{h}",
                                    name=f"z{j % 4}_{h}")
                        nc.vector._custom_dve(SOFT, out=z[:], in0=ps[:],
                                              in1=lam[h][j][:],
                                              s0=(L if t == 0 else 1.0))
                        znew[h] = z
                    zprev[j] = znew
            for j in jset:
                js = slice(j * 512, (j + 1) * 512)
                for h in range(2):
                    nc.sync.dma_start(zo_d[128 * h:128 * h + 128, js],
                                      zprev[j][h][:].bitcast(f32))
                psr = pp.tile([64, 512], f32, tag="ps", name="psr")
                for h in range(2):
                    nc.tensor.matmul(psr[:], dctt[:, h, :], zprev[j][h][:],
                                     start=(h == 0), stop=(h == 1))
                rc = ev.tile([64, 512], f32, tag="rc", name="rc", bufs=2)
                nc.scalar.copy(rc[:], psr[:])
                nc.sync.dma_start(ro_d[:, js], rc[:])

        zp_cm.__exit__(None, None, None)
        late_cm.__exit__(None, None, None)
        pp_cm.__exit__(None, None, None)
        ev_cm.__exit__(None, None, None)
        wp_cm.__exit__(None, None, None)

    nc.compile()
    return nc


def _host_prep(inputs):
    b16 = ml_dtypes.bfloat16
    cw = np.ascontiguousarray(inputs["conv_w"], dtype=np.float32)
    cwp = np.zeros((128, 2, 3, 128), np.float32)
    cws = np.zeros((64, 2, 3, 128), np.float32)
    for h in range(2):
        for dc in range(3):
            cwp[0:64, h, dc, :] = cw[128 * h:128 * h + 128, :, 0, dc].T
            cwp[64:128, h, dc, :] = cw[128 * h:128 * h + 128, :, 1, dc].T
            cws[:, h, dc, :] = cw[128 * h:128 * h + 128, :, 2, dc].T
    b2 = np.ascontiguousarray(
        np.asarray(inputs["conv_b"], np.float32).reshape(2, 128).T)

    def res_pack(w1, w2):
        w1 = np.asarray(w1, np.float32)
        r1 = np.zeros((128, 9, 2, 128), np.float32)
        for kh in range(3):
            for kw in range(3):
                for h in range(2):
                    r1[:, kh * 3 + kw, h, :] = w1[:, 128 * h:128 * h + 128, kh, kw].T
        r2 = np.ascontiguousarray(np.asarray(w2, np.float32)[:, :, 0, 0].T)
        return r1.astype(b16), r2.astype(b16)

    rw1a, rw2a = res_pack(inputs["res1_w1"], inputs["res1_w2"])
    rw1b, rw2b = res_pack(inputs["res2_w1"], inputs["res2_w2"])
    caw1 = np.ascontiguousarray(
        np.asarray(inputs["ca_w1"], np.float32).T.reshape(2, 128, 16)
        .transpose(1, 0, 2))
    caw2 = np.ascontiguousarray(np.asarray(inputs["ca_w2"], np.float32).T)
    D = np.asarray(inputs["Dict"], np.float32)
    L = float(np.asarray(inputs["L"]))
    S = np.ascontiguousarray(
        (np.eye(256, dtype=np.float32) - (D.T @ D) / np.float32(L)).T
        .reshape(2, 128, 256).transpose(1, 0, 2))
    dctt = np.ascontiguousarray(D.T.reshape(2, 128, 64).transpose(1, 0, 2))
    return dict(cwp=cwp, cws=cws, b2=b2,
                rw1a=rw1a, rw2a=rw2a, rw1b=rw1b, rw2b=rw2b, caw1=caw1,
                caw2=caw2, S=S, Dct=D, DctT=dctt,
                ident=np.eye(128, dtype=np.float32)), L


def kernel(**inputs):
    num_iters = int(np.asarray(inputs["num_iters"]))
    sa_w = np.asarray(inputs["sa_w"], np.float32)
    weights, L = _host_prep(inputs)
    nc = build_nc(num_iters, L, sa_w)
    x = np.ascontiguousarray(np.asarray(inputs["x"], np.float32))
    in_maps = [dict(weights, x=x[b]) for b in range(8)]
    res = run_bass_kernel_spmd(nc, in_maps, core_ids=list(range(8)))
    z = np.stack([res.results[b]["zo"].reshape(256, 64, 64) for b in range(8)])
    rec = np.stack([res.results[b]["ro"].reshape(64, 64, 64) for b in range(8)])
    return (z, rec, np.asarray(inputs["Dict"], np.float32))


if __name__ == "__main__":
    d = np.load("/root/problem/ref_cache.npz")
    ins = {k: d[k] for k in ["x", "conv_w", "conv_b", "res1_w1", "res1_w2",
                             "res2_w1", "res2_w2", "ca_w1", "ca_w2", "sa_w",
                             "Dict", "L", "num_iters"]}
    out = kernel(**ins)
    for i, name in enumerate(["z", "recon", "Dict"]):
        ref = d[f"out{i}"]
        got = out[i]
        num = np.abs(got - ref).max()
        den = np.abs(ref).max()
        print(f"{name}: absmax diff {num:.3e}  scale {den:.3e}  rel {num/den:.3e}")
